# revision 36
# baseline (speedup 1.0000x reference)
"""HGT-style heterogeneous graph message passing on 8 Trainium2 cores.

Strategy:
 - Host folds the per-(head, etype) relation transforms into per-(ntype, etype)
   64x128 weight matrices:  a_e = <k'_src, q_dst>  with
   k' = x @ Wk[nt] @ blockdiag_h(A A^T * pri / sqrt(d)),  m = x @ Wv[nt] @ blockdiag_h(M).
 - dst nodes are sharded across the 8 cores round-robin by degree rank, so all
   segment ops (softmax max/sum, weighted aggregation) become dense row
   reductions over degree-sorted [128, W_t] tiles.  No collectives.
 - Each core computes a deduplicated (src, etype) pair table [rows, 128] =
   [k' | m] on device (PE matmuls), then dma_gather's the rows of its slots.
"""

import sys

sys.path.insert(0, "/opt/trn_rl_repo")

import numpy as np

N, E = 40000, 640000
IN, H, HS = 64, 4, 16
NT, ET = 4, 8
D = H * HS  # 64
C = 8  # cores
NL = 5120  # padded local nodes per core
NTILES = NL // 128  # 40
NEG = -1.0e30

_cache = {}
LAST_RESULT = None  # BassKernelResults of the most recent run (for test harness)


def _host_prep(x, ntype, etype, src, dst):
    """Returns per-core input arrays + structural constants."""
    x = np.ascontiguousarray(np.asarray(x, dtype=np.float32))
    nt_ = np.asarray(ntype).astype(np.int64)
    et_ = np.asarray(etype).astype(np.int64)
    src = np.asarray(src).astype(np.int64)
    dst = np.asarray(dst).astype(np.int64)

    deg = np.bincount(dst, minlength=N)
    order = np.argsort(-deg, kind="stable")
    ranks = np.empty(N, dtype=np.int64)
    ranks[order] = np.arange(N)
    core_of_node = ranks % C
    local_of_node = ranks // C

    # tile widths (shared across cores): tile t covers global ranks [1024t, 1024(t+1))
    W = np.zeros(NTILES, dtype=np.int64)
    deg_by_rank = deg[order]
    for t in range(NTILES):
        lo, hi = t * 1024, min((t + 1) * 1024, N)
        W[t] = max(int(deg_by_rank[lo:hi].max()) if hi > lo else 1, 1)

    percore = []
    for c in range(C):
        ei = np.nonzero(core_of_node[dst] == c)[0]
        ld = local_of_node[dst[ei]]
        o = np.argsort(ld, kind="stable")
        percore.append((ei[o], ld[o]))

    # table chunks: tiny leading chunks so the first gather starts early,
    # big trailing chunks to limit per-group 64-row alignment padding.
    # Each chunk's padded pair count must stay < 32000 (int16 gather idx).
    patterns = [[2, 3, 5, 5, 5, 5, 5, 5, 5], [5] * 8, [4] * 10, [2] * 20]
    for pat in patterns:
        assert sum(pat) == NTILES
        NCH = len(pat)
        tile_chunk = np.repeat(np.arange(NCH), pat)  # [NTILES] -> chunk id
        cnts = np.zeros((C, NCH, NT * ET), dtype=np.int64)
        pair_data = []
        for c in range(C):
            ei, ld = percore[c]
            ch_of = tile_chunk[ld // 128]
            key = src[ei] * ET + et_[ei]
            chunk_pairs = []
            for h in range(NCH):
                uk = np.unique(key[ch_of == h])  # sorted keys
                g = nt_[uk // ET] * ET + (uk % ET)
                np.add.at(cnts[c, h], g, 1)
                chunk_pairs.append((uk, g))
            pair_data.append(chunk_pairs)
        # 64-row group alignment: matmul output base partition must be 0/64
        R = 64 * ((cnts.max(axis=0) + 63) // 64)  # [NCH, 32]
        CHRs = 128 * ((R.sum(axis=1) + 127) // 128)
        if CHRs.max() < 32000:
            break
    else:
        raise RuntimeError("could not chunk tables under int16 limit")

    gbase = np.zeros((NCH, NT * ET), dtype=np.int64)
    for h in range(NCH):
        gbase[h] = np.concatenate(([0], np.cumsum(R[h])[:-1]))
    CHRs = [int(v) for v in CHRs]
    RB = np.concatenate(([0], np.cumsum(CHRs)[:-1])).astype(np.int64)
    RPtot = int(sum(CHRs))

    IDX8 = (8 * W).astype(np.int64)
    off8 = np.concatenate(([0], np.cumsum(IDX8)[:-1]))
    offw = np.concatenate(([0], np.cumsum(W)[:-1]))
    IDXW = int(IDX8.sum())
    ABW = int(W.sum())

    cores = []
    own_nodes = np.full((C, NL), -1, dtype=np.int64)
    for c in range(C):
        ei, ld = percore[c]
        etile = ld // 128
        ch_of = tile_chunk[etile]
        key = src[ei] * ET + et_[ei]

        ownc = order[c::C]
        own_nodes[c, : len(ownc)] = ownc

        rowid_of_edge = np.zeros(len(ei), dtype=np.int64)
        xp_node = np.full(RPtot, -1, dtype=np.int64)
        for h in range(NCH):
            uk, g = pair_data[c][h]  # uk sorted by key; g aligned
            po = np.argsort(g, kind="stable")
            gs = g[po]
            base_in_g = np.concatenate(
                ([0], np.cumsum(np.bincount(gs, minlength=NT * ET))[:-1])
            )
            rows_po = gbase[h][gs] + (np.arange(len(uk)) - base_in_g[gs])
            row_of_uk = np.empty(len(uk), dtype=np.int64)
            row_of_uk[po] = rows_po
            xp_node[RB[h] + row_of_uk] = uk // ET
            sel = np.nonzero(ch_of == h)[0]
            rowid_of_edge[sel] = row_of_uk[np.searchsorted(uk, key[sel])]

        import ml_dtypes

        xpT = np.zeros((IN, RPtot), dtype=ml_dtypes.bfloat16)
        valid = xp_node >= 0
        xpT[:, valid] = x[xp_node[valid]].T.astype(ml_dtypes.bfloat16)

        cnt = np.bincount(ld, minlength=NL)
        starts = np.concatenate(([0], np.cumsum(cnt)[:-1]))
        jpos = np.arange(len(ei)) - starts[ld]
        p_of = ld % 128

        kmidx = np.zeros((128, IDXW), dtype=np.int16)
        abias = np.full((128, ABW), NEG, dtype=np.float32)
        for t in range(NTILES):
            wt = int(W[t])
            sel = np.nonzero(etile == t)[0]
            M = np.zeros((128, wt), dtype=np.int16)
            M[p_of[sel], jpos[sel]] = rowid_of_edge[sel].astype(np.int16)
            idsl = M.T.ravel()  # list position k = j*128 + p
            wrapped = idsl.reshape(8 * wt, 16).T
            kmidx[:, int(off8[t]) : int(off8[t]) + 8 * wt] = np.tile(wrapped, (8, 1))
            B = np.full((128, wt), NEG, dtype=np.float32)
            B[p_of[sel], jpos[sel]] = 0.0
            abias[:, int(offw[t]) : int(offw[t]) + wt] = B

        x4T = np.zeros((NT * IN, NL), dtype=np.float32)
        nreal = len(ownc)
        ntc = nt_[ownc]
        xo = x[ownc]
        for t4 in range(NT):
            m4 = ntc == t4
            x4T[t4 * IN : (t4 + 1) * IN, :nreal][:, m4] = xo[m4].T

        oneh = np.zeros((NL, NT), dtype=np.float32)
        oneh[np.arange(nreal), ntc] = 1.0

        cores.append(dict(xpT=xpT, x4T=x4T, oneh=oneh, kmidx=kmidx, abias=abias))

    consts = dict(
        W=W, WMAX=int(W.max()), NCH=NCH, tile_chunk=tile_chunk, R=R, gbase=gbase,
        CHRs=CHRs, RB=RB, RPtot=RPtot, IDXW=IDXW, ABW=ABW, off8=off8, offw=offw,
        own_nodes=own_nodes, deg=deg,
    )
    return cores, consts


def _fold_weights(Wk, Wq, Wv, Wa, rel_att, rel_msg, rel_pri):
    Wk = np.asarray(Wk, np.float64)
    Wq = np.asarray(Wq, np.float64)
    Wv = np.asarray(Wv, np.float64)
    Wa = np.asarray(Wa, np.float64)
    rel_att = np.asarray(rel_att, np.float64)
    rel_msg = np.asarray(rel_msg, np.float64)
    rel_pri = np.asarray(rel_pri, np.float64)
    sd = float(np.sqrt(np.float32(HS)))

    wkm = np.zeros((IN, NT * ET, 2, D), np.float64)
    for nt in range(NT):
        for et in range(ET):
            Batt = np.zeros((D, D))
            Bmsg = np.zeros((D, D))
            for h in range(H):
                A = rel_att[h, et]
                Batt[h * HS : (h + 1) * HS, h * HS : (h + 1) * HS] = (
                    A @ A.T * rel_pri[h, et] / sd
                )
                Bmsg[h * HS : (h + 1) * HS, h * HS : (h + 1) * HS] = rel_msg[h, et]
            g = nt * ET + et
            wkm[:, g, 0] = Wk[nt] @ Batt
            wkm[:, g, 1] = Wv[nt] @ Bmsg
    import ml_dtypes

    wkm = wkm.reshape(IN, NT * ET * 2 * D).astype(ml_dtypes.bfloat16)
    wq4 = np.concatenate([Wq[t] for t in range(NT)], axis=0).astype(np.float32)
    wa_all = np.concatenate([Wa[t] for t in range(NT)], axis=1).astype(np.float32)
    return wkm, wq4, wa_all


def _build_program(consts):
    import concourse.mybir as mybir
    import concourse.tile as tile
    from concourse import bacc
    from concourse.masks import make_identity

    f32 = mybir.dt.float32
    bf16 = mybir.dt.bfloat16
    i16 = mybir.dt.int16
    W = consts["W"]
    WMAX = consts["WMAX"]
    NCH, tile_chunk = consts["NCH"], consts["tile_chunk"]
    chunk_tiles = [
        [t for t in range(NTILES) if tile_chunk[t] == h] for h in range(NCH)
    ]
    R, gbase, CHRs, RB = consts["R"], consts["gbase"], consts["CHRs"], consts["RB"]
    RPtot, IDXW, ABW = consts["RPtot"], consts["IDXW"], consts["ABW"]
    off8, offw = consts["off8"], consts["offw"]

    nc = bacc.Bacc("TRN2", target_bir_lowering=False, debug=False, num_devices=C)

    xpT = nc.dram_tensor("xpT", [IN, RPtot], bf16, kind="ExternalInput").ap()
    wkm = nc.dram_tensor("wkm", [IN, NT * ET * 2 * D], bf16, kind="ExternalInput").ap()
    x4T = nc.dram_tensor("x4T", [NT * IN, NL], f32, kind="ExternalInput").ap()
    wq4 = nc.dram_tensor("wq4", [NT * IN, D], f32, kind="ExternalInput").ap()
    wa = nc.dram_tensor("wa", [D, NT * D], f32, kind="ExternalInput").ap()
    oneh = nc.dram_tensor("oneh", [NL, NT], f32, kind="ExternalInput").ap()
    kmidx = nc.dram_tensor("kmidx", [128, IDXW], i16, kind="ExternalInput").ap()
    abias = nc.dram_tensor("abias", [128, ABW], f32, kind="ExternalInput").ap()
    outp = nc.dram_tensor("outp", [NL, D], f32, kind="ExternalOutput").ap()
    kmtab = [
        nc.dram_tensor(f"kmtab{h}", [CHRs[h], 2 * D], f32, kind="Internal").ap()
        for h in range(NCH)
    ]

    with tile.TileContext(nc) as tc:
        with tc.tile_pool(name="const", bufs=1) as constp, \
             tc.tile_pool(name="stage", bufs=2) as stage, \
             tc.tile_pool(name="work", bufs=3) as work, \
             tc.tile_pool(name="npsum", bufs=2, space="PSUM") as npsum, \
             tc.tile_pool(name="qpsum", bufs=2, space="PSUM") as qpsum, \
             tc.tile_pool(name="opsum", bufs=2, space="PSUM") as opsum:

            # ---- persistent constants ----
            # only wkm is needed for the chunk-0 table build; the rest load
            # via late_consts() after chunk 0's slabs are queued, so the first
            # table build (which gates the first gather) hits DMA sooner.
            wkm_s = constp.tile([IN, NT * ET * 2 * D], bf16, name="wkm_s", tag="wkm_s")
            nc.sync.dma_start(out=wkm_s[:], in_=wkm[:, :])
            wq4_s = constp.tile([128, 2 * D], f32, name="wq4_s", tag="wq4_s")
            wa_s = constp.tile([D, NT * D], f32, name="wa_s", tag="wa_s")
            oneh_s = constp.tile([128, NTILES * NT], f32, name="oneh_s", tag="oneh_s")
            kmidx_s = constp.tile([128, IDXW], i16, name="kmidx_s", tag="kmidx_s")
            abias_s = constp.tile([128, ABW], f32, name="abias_s", tag="abias_s")
            ident = constp.tile([128, 128], f32, name="ident", tag="ident")
            qall = constp.tile([128, NTILES * D], f32, name="qall", tag="qall")

            def late_consts():
                for k in range(2):
                    nc.sync.dma_start(
                        out=wq4_s[:, k * D : (k + 1) * D],
                        in_=wq4[k * 128 : (k + 1) * 128, :],
                    )
                nc.sync.dma_start(out=wa_s[:], in_=wa[:, :])
                nc.sync.dma_start(
                    out=oneh_s[:].rearrange("p (t f) -> p t f", t=NTILES),
                    in_=oneh[:, :].rearrange("(t p) f -> p t f", p=128),
                )
                nc.sync.dma_start(out=kmidx_s[:], in_=kmidx[:, :])
                nc.sync.dma_start(out=abias_s[:], in_=abias[:, :])
                make_identity(nc, ident[:])

            # ---- Q phase (emitted per chunk, 4 node-tiles per load) ----
            def q_tiles(ts):
                for g0 in range(0, len(ts), 4):
                    grp = ts[g0 : g0 + 4]
                    t0, ng = grp[0], len(grp)
                    x4_s = stage.tile([128, 2, 512], f32, name=f"x4_{t0}", tag="x4")
                    nc.sync.dma_start(
                        out=x4_s[:, :, : ng * 128],
                        in_=x4T[:, t0 * 128 : (t0 + ng) * 128].rearrange(
                            "(k p) n -> p k n", p=128
                        ),
                    )
                    for i in range(ng):
                        t = t0 + i
                        q_p = qpsum.tile(
                            [128, D], f32, space="PSUM", name=f"q_p{t}", tag="q_p"
                        )
                        for k in range(2):
                            nc.tensor.matmul(
                                q_p[:],
                                lhsT=x4_s[:, k, i * 128 : (i + 1) * 128],
                                rhs=wq4_s[:, k * D : (k + 1) * D],
                                start=(k == 0),
                                stop=(k == 1),
                            )
                        nc.any.tensor_copy(out=qall[:, t * D : (t + 1) * D], in_=q_p[:])

            # ---- node/pair-table phase per chunk ----
            # Groups are packed back-to-back (no 128-row alignment); each
            # 128-row output tile may span several (ntype, etype) groups, so
            # it gets one partial-M matmul per overlapped group.
            def node_chunk_emitters(h):
                bounds = []  # (start_row, end_row, g) for nonempty groups
                for g in range(NT * ET):
                    if int(R[h, g]) > 0:
                        bounds.append((int(gbase[h, g]), int(gbase[h, g]) + int(R[h, g]), g))
                GT = bounds[-1][1]  # real rows (64-aligned)
                n_tiles = (GT + 127) // 128
                SLAB = 16
                emitters = []
                for s0 in range(0, n_tiles, SLAB):
                    emitters.append(
                        lambda s0=s0: node_slab(h, bounds, GT, n_tiles, SLAB, s0)
                    )
                return emitters

            def node_slab(h, bounds, GT, n_tiles, SLAB, s0):
                if True:
                    nb = min(SLAB, n_tiles - s0)
                    row0 = s0 * 128
                    rows = min(GT, (s0 + nb) * 128) - row0
                    lhs_s = stage.tile(
                        [IN, SLAB * 128], bf16, name=f"lhs_{h}_{s0}", tag="lhs"
                    )
                    nc.sync.dma_start(
                        out=lhs_s[:, :rows],
                        in_=xpT[:, int(RB[h]) + row0 : int(RB[h]) + row0 + rows],
                    )
                    slab = stage.tile(
                        [128, SLAB, 2 * D], f32, name=f"slab_{h}_{s0}", tag="slab"
                    )
                    for i in range(0, nb, 4):
                        nn = min(4, nb - i)
                        km_p = npsum.tile(
                            [128, 512], f32, space="PSUM", name=f"km_p{h}_{s0}_{i}", tag="km_p"
                        )
                        covers = []
                        for j in range(nn):
                            t0 = row0 + (i + j) * 128  # tile's first table row
                            covers.append(min(128, GT - t0))
                            for gs, ge, g in bounds:
                                lo, hi = max(gs, t0), min(ge, t0 + 128)
                                if lo >= hi:
                                    continue
                                nc.tensor.matmul(
                                    km_p[lo - t0 : hi - t0, j * 128 : (j + 1) * 128],
                                    lhsT=lhs_s[:, (i + j) * 128 + lo - t0 : (i + j) * 128 + hi - t0],
                                    rhs=wkm_s[:, g * 128 : (g + 1) * 128],
                                    start=True,
                                    stop=True,
                                )
                        if covers[-1] == 128:
                            nc.any.tensor_copy(
                                out=slab[:, i : i + nn],
                                in_=km_p[:].rearrange("p (a d) -> p a d", a=4)[:, :nn],
                            )
                        else:
                            for j in range(nn):
                                nc.any.tensor_copy(
                                    out=slab[: covers[j], i + j],
                                    in_=km_p[: covers[j], j * 128 : (j + 1) * 128],
                                )
                    nf = rows // 128  # full tiles in this slab
                    if nf:
                        nc.sync.dma_start(
                            out=kmtab[h][row0 : row0 + nf * 128, :].rearrange(
                                "(a p) d -> p a d", p=128
                            ),
                            in_=slab[:, :nf],
                        )
                    if rows % 128:
                        pr = rows % 128
                        nc.sync.dma_start(
                            out=kmtab[h][row0 + nf * 128 : row0 + rows, :],
                            in_=slab[:pr, nf],
                        )

            # ---- phase 3: per node-tile softmax + aggregation ----
            def p3_tile(t):
                h = int(tile_chunk[t])
                wt = int(W[t])
                n_idx = 128 * wt
                o8, ow = int(off8[t]), int(offw[t])
                gt = work.tile([128, WMAX, 2 * D], f32, name=f"gt{t}", tag="gt")
                nc.gpsimd.dma_gather(
                    out_ap=gt[:, :wt],
                    in_ap=kmtab[h][:, :],
                    idxs_ap=kmidx_s[:, o8 : o8 + 8 * wt],
                    num_idxs=n_idx,
                    num_idxs_reg=n_idx,
                    elem_size=2 * D,
                    single_packet=False,
                )
                aprod = work.tile([128, WMAX, D], f32, name=f"aprod{t}", tag="aprod")
                qb = qall[:, t * D : (t + 1) * D].unsqueeze(1).to_broadcast([128, wt, D])
                nc.vector.tensor_tensor(
                    out=aprod[:, :wt], in0=gt[:, :wt, :D], in1=qb, op=mybir.AluOpType.mult
                )
                am = work.tile([128, H, WMAX], f32, name=f"am{t}", tag="am")
                nc.vector.tensor_reduce(
                    out=am[:, :, :wt],
                    in_=aprod[:, :wt].rearrange("p w (h d) -> p h w d", h=H),
                    axis=mybir.AxisListType.X,
                    op=mybir.AluOpType.add,
                )
                amb = work.tile([128, H, WMAX], f32, name=f"amb{t}", tag="amb")
                bb = abias_s[:, ow : ow + wt].unsqueeze(1).to_broadcast([128, H, wt])
                nc.vector.tensor_tensor(
                    out=amb[:, :, :wt], in0=am[:, :, :wt], in1=bb, op=mybir.AluOpType.add
                )
                # softmax without max-subtraction: |a| is bounded well below
                # f32 exp overflow, and pads carry a -1e30 bias -> exp == 0.
                ex = work.tile([128, H, WMAX], f32, name=f"ex{t}", tag="ex")
                nc.scalar.activation(
                    out=ex[:, :, :wt], in_=amb[:, :, :wt],
                    func=mybir.ActivationFunctionType.Exp,
                )
                den = work.tile([128, H], f32, name=f"den{t}", tag="den")
                nc.vector.tensor_reduce(
                    out=den[:], in_=ex[:, :, :wt],
                    axis=mybir.AxisListType.X, op=mybir.AluOpType.add,
                )
                rden = work.tile([128, H], f32, name=f"rden{t}", tag="rden")
                nc.vector.reciprocal(out=rden[:], in_=den[:])
                mprod = work.tile([128, H, HS, WMAX], f32, name=f"mprod{t}", tag="mprod")
                mpart = gt[:, :wt, D:].rearrange("p w (h d) -> p h d w", h=H)
                ab2 = ex[:, :, :wt].unsqueeze(2).to_broadcast([128, H, HS, wt])
                nc.vector.tensor_tensor(
                    out=mprod[:, :, :, :wt], in0=mpart, in1=ab2, op=mybir.AluOpType.mult
                )
                hm = work.tile([128, D], f32, name=f"hm{t}", tag="hm")
                nc.vector.tensor_reduce(
                    out=hm[:].rearrange("p (h d) -> p h d", h=H),
                    in_=mprod[:, :, :, :wt],
                    axis=mybir.AxisListType.X,
                    op=mybir.AluOpType.add,
                )
                hm2 = work.tile([128, D], f32, name=f"hm2{t}", tag="hm2")
                nc.vector.tensor_tensor(
                    out=hm2[:].rearrange("p (h d) -> p h d", h=H),
                    in0=hm[:].rearrange("p (h d) -> p h d", h=H),
                    in1=rden[:].unsqueeze(2).to_broadcast([128, H, HS]),
                    op=mybir.AluOpType.mult,
                )
                tp = opsum.tile([128, 128], f32, space="PSUM", name=f"tp{t}", tag="tp")
                nc.tensor.transpose(out=tp[:D, :], in_=hm2[:], identity=ident[:])
                hT = work.tile([D, 128], f32, name=f"hT{t}", tag="hT")
                nc.any.tensor_copy(out=hT[:], in_=tp[:D, :])
                o4 = opsum.tile([128, NT * D], f32, space="PSUM", name=f"o4_{t}", tag="o4")
                nc.tensor.matmul(o4[:], lhsT=hT[:], rhs=wa_s[:], start=True, stop=True)
                osel = work.tile([128, NT * D], f32, name=f"osel{t}", tag="osel")
                ohb = (
                    oneh_s[:]
                    .rearrange("p (t f) -> p t f", t=NTILES)[:, t]
                    .unsqueeze(1)
                    .to_broadcast([128, D, NT])
                )
                nc.vector.tensor_tensor(
                    out=osel[:].rearrange("p (t d) -> p d t", t=NT),
                    in0=o4[:].rearrange("p (t d) -> p d t", t=NT),
                    in1=ohb,
                    op=mybir.AluOpType.mult,
                )
                ot = work.tile([128, D], f32, name=f"ot{t}", tag="ot")
                nc.vector.tensor_reduce(
                    out=ot[:],
                    in_=osel[:].rearrange("p (t d) -> p d t", t=NT),
                    axis=mybir.AxisListType.X,
                    op=mybir.AluOpType.add,
                )
                nc.sync.dma_start(out=outp[t * 128 : (t + 1) * 128, :], in_=ot[:])

            # emission order = scheduler priority: build chunk 0's table first,
            # then interleave later chunks' table slabs with phase 3 of the
            # already-built chunks so DMA/PE/DVE overlap across phases.
            import os

            mode = os.environ.get("GNN_EMIT", "chunk")
            if mode == "chunk":
                for em in node_chunk_emitters(0):
                    em()
                late_consts()
                q_tiles(chunk_tiles[0])
                for h in range(1, NCH):
                    for em in node_chunk_emitters(h):
                        em()
                    q_tiles(chunk_tiles[h])
                    for t in chunk_tiles[h - 1]:
                        p3_tile(t)
                for t in chunk_tiles[NCH - 1]:
                    p3_tile(t)
            else:  # interleave
                for em in node_chunk_emitters(0):
                    em()
                late_consts()
                q_tiles(chunk_tiles[0])
                for h in range(1, NCH):
                    q_tiles(chunk_tiles[h])
                    slabs = node_chunk_emitters(h)
                    tiles = chunk_tiles[h - 1]
                    ns, ntl = len(slabs), len(tiles)
                    si = ti = 0
                    while si < ns or ti < ntl:
                        take = (si + 1) * ntl <= (ti + 1) * ns
                        if si < ns and (take or ti >= ntl):
                            slabs[si]()
                            si += 1
                        else:
                            p3_tile(tiles[ti])
                            ti += 1
                for t in chunk_tiles[NCH - 1]:
                    p3_tile(t)

    nc.compile()
    return nc


def kernel(x, ntype, etype, src, dst, Wk, Wq, Wv, Wa, rel_att, rel_msg, rel_pri):
    from concourse import bass_utils

    cores, consts = _host_prep(x, ntype, etype, src, dst)
    wkm, wq4, wa_all = _fold_weights(Wk, Wq, Wv, Wa, rel_att, rel_msg, rel_pri)

    struct_sig = (
        tuple(consts["W"].tolist()),
        consts["NCH"],
        tuple(consts["CHRs"]),
        tuple(consts["R"].ravel().tolist()),
    )
    if "prog" not in _cache or _cache["prog"][0] != struct_sig:
        _cache["prog"] = (struct_sig, _build_program(consts))
    nc = _cache["prog"][1]

    in_maps = [
        dict(
            xpT=d["xpT"], wkm=wkm, x4T=d["x4T"], wq4=wq4, wa=wa_all,
            oneh=d["oneh"], kmidx=d["kmidx"], abias=d["abias"],
        )
        for d in cores
    ]
    res = bass_utils.run_bass_kernel_spmd(nc, in_maps, core_ids=list(range(C)))
    global LAST_RESULT
    LAST_RESULT = res

    out = np.zeros((N, D), dtype=np.float32)
    own = consts["own_nodes"]
    for c in range(C):
        oc = res.results[c]["outp"]
        m = own[c] >= 0
        out[own[c][m]] = oc[m]
    out[consts["deg"] == 0] = 0.0
    return out



# revision 37
# speedup vs baseline: 1.1310x; 1.1310x over previous
"""HGT-style heterogeneous graph message passing on 8 Trainium2 cores.

Strategy:
 - Host folds the per-(head, etype) relation transforms into per-(ntype, etype)
   64x128 weight matrices:  a_e = <k'_src, q_dst>  with
   k' = x @ Wk[nt] @ blockdiag_h(A A^T * pri / sqrt(d)),  m = x @ Wv[nt] @ blockdiag_h(M).
 - dst nodes are sharded across the 8 cores round-robin by degree rank, so all
   segment ops (softmax max/sum, weighted aggregation) become dense row
   reductions over degree-sorted [128, W_t] tiles.  No collectives.
 - Each core computes a deduplicated (src, etype) pair table [rows, 128] =
   [k' | m] on device (PE matmuls), then dma_gather's the rows of its slots.
"""

import sys

sys.path.insert(0, "/opt/trn_rl_repo")

import numpy as np

N, E = 40000, 640000
IN, H, HS = 64, 4, 16
NT, ET = 4, 8
D = H * HS  # 64
C = 8  # cores
NL = 5120  # padded local nodes per core
NTILES = NL // 128  # 40
NEG = -1.0e30

_cache = {}
LAST_RESULT = None  # BassKernelResults of the most recent run (for test harness)


def _host_prep(x, ntype, etype, src, dst):
    """Returns per-core input arrays + structural constants."""
    x = np.ascontiguousarray(np.asarray(x, dtype=np.float32))
    nt_ = np.asarray(ntype).astype(np.int64)
    et_ = np.asarray(etype).astype(np.int64)
    src = np.asarray(src).astype(np.int64)
    dst = np.asarray(dst).astype(np.int64)

    deg = np.bincount(dst, minlength=N)
    order = np.argsort(-deg, kind="stable")
    ranks = np.empty(N, dtype=np.int64)
    ranks[order] = np.arange(N)
    core_of_node = ranks % C
    local_of_node = ranks // C

    # tile widths (shared across cores): tile t covers global ranks [1024t, 1024(t+1))
    W = np.zeros(NTILES, dtype=np.int64)
    deg_by_rank = deg[order]
    for t in range(NTILES):
        lo, hi = t * 1024, min((t + 1) * 1024, N)
        W[t] = max(int(deg_by_rank[lo:hi].max()) if hi > lo else 1, 1)

    percore = []
    for c in range(C):
        ei = np.nonzero(core_of_node[dst] == c)[0]
        ld = local_of_node[dst[ei]]
        o = np.argsort(ld, kind="stable")
        percore.append((ei[o], ld[o]))

    # table chunks: tiny leading chunks so the first gather starts early,
    # big trailing chunks to limit per-group 64-row alignment padding.
    # Each chunk's padded pair count must stay < 32000 (int16 gather idx).
    patterns = [[5] * 8, [4] * 10, [2] * 20]
    for pat in patterns:
        assert sum(pat) == NTILES
        NCH = len(pat)
        tile_chunk = np.repeat(np.arange(NCH), pat)  # [NTILES] -> chunk id
        cnts = np.zeros((C, NCH, NT * ET), dtype=np.int64)
        pair_data = []
        for c in range(C):
            ei, ld = percore[c]
            ch_of = tile_chunk[ld // 128]
            key = src[ei] * ET + et_[ei]
            chunk_pairs = []
            for h in range(NCH):
                uk = np.unique(key[ch_of == h])  # sorted keys
                g = nt_[uk // ET] * ET + (uk % ET)
                np.add.at(cnts[c, h], g, 1)
                chunk_pairs.append((uk, g))
            pair_data.append(chunk_pairs)
        # 64-row group alignment: matmul output base partition must be 0/64
        R = 64 * ((cnts.max(axis=0) + 63) // 64)  # [NCH, 32]
        CHRs = 128 * ((R.sum(axis=1) + 127) // 128)
        if CHRs.max() < 32000:
            break
    else:
        raise RuntimeError("could not chunk tables under int16 limit")

    gbase = np.zeros((NCH, NT * ET), dtype=np.int64)
    for h in range(NCH):
        gbase[h] = np.concatenate(([0], np.cumsum(R[h])[:-1]))
    CHRs = [int(v) for v in CHRs]
    RB = np.concatenate(([0], np.cumsum(CHRs)[:-1])).astype(np.int64)
    RPtot = int(sum(CHRs))

    IDX8 = (8 * W).astype(np.int64)
    off8 = np.concatenate(([0], np.cumsum(IDX8)[:-1]))
    offw = np.concatenate(([0], np.cumsum(W)[:-1]))
    IDXW = int(IDX8.sum())
    ABW = int(W.sum())

    cores = []
    own_nodes = np.full((C, NL), -1, dtype=np.int64)
    for c in range(C):
        ei, ld = percore[c]
        etile = ld // 128
        ch_of = tile_chunk[etile]
        key = src[ei] * ET + et_[ei]

        ownc = order[c::C]
        own_nodes[c, : len(ownc)] = ownc

        rowid_of_edge = np.zeros(len(ei), dtype=np.int64)
        xp_node = np.full(RPtot, -1, dtype=np.int64)
        for h in range(NCH):
            uk, g = pair_data[c][h]  # uk sorted by key; g aligned
            po = np.argsort(g, kind="stable")
            gs = g[po]
            base_in_g = np.concatenate(
                ([0], np.cumsum(np.bincount(gs, minlength=NT * ET))[:-1])
            )
            rows_po = gbase[h][gs] + (np.arange(len(uk)) - base_in_g[gs])
            row_of_uk = np.empty(len(uk), dtype=np.int64)
            row_of_uk[po] = rows_po
            xp_node[RB[h] + row_of_uk] = uk // ET
            sel = np.nonzero(ch_of == h)[0]
            rowid_of_edge[sel] = row_of_uk[np.searchsorted(uk, key[sel])]

        import ml_dtypes

        xpT = np.zeros((IN, RPtot), dtype=ml_dtypes.bfloat16)
        valid = xp_node >= 0
        xpT[:, valid] = x[xp_node[valid]].T.astype(ml_dtypes.bfloat16)

        cnt = np.bincount(ld, minlength=NL)
        starts = np.concatenate(([0], np.cumsum(cnt)[:-1]))
        jpos = np.arange(len(ei)) - starts[ld]
        p_of = ld % 128

        kmidx = np.zeros((128, IDXW), dtype=np.int16)
        abias = np.full((128, ABW), NEG, dtype=np.float32)
        for t in range(NTILES):
            wt = int(W[t])
            sel = np.nonzero(etile == t)[0]
            M = np.zeros((128, wt), dtype=np.int16)
            M[p_of[sel], jpos[sel]] = rowid_of_edge[sel].astype(np.int16)
            idsl = M.T.ravel()  # list position k = j*128 + p
            wrapped = idsl.reshape(8 * wt, 16).T
            kmidx[:, int(off8[t]) : int(off8[t]) + 8 * wt] = np.tile(wrapped, (8, 1))
            B = np.full((128, wt), NEG, dtype=np.float32)
            B[p_of[sel], jpos[sel]] = 0.0
            abias[:, int(offw[t]) : int(offw[t]) + wt] = B

        x4T = np.zeros((NT * IN, NL), dtype=np.float32)
        nreal = len(ownc)
        ntc = nt_[ownc]
        xo = x[ownc]
        for t4 in range(NT):
            m4 = ntc == t4
            x4T[t4 * IN : (t4 + 1) * IN, :nreal][:, m4] = xo[m4].T

        oneh = np.zeros((NL, NT), dtype=np.float32)
        oneh[np.arange(nreal), ntc] = 1.0

        cores.append(dict(xpT=xpT, x4T=x4T, oneh=oneh, kmidx=kmidx, abias=abias))

    consts = dict(
        W=W, WMAX=int(W.max()), NCH=NCH, tile_chunk=tile_chunk, R=R, gbase=gbase,
        CHRs=CHRs, RB=RB, RPtot=RPtot, IDXW=IDXW, ABW=ABW, off8=off8, offw=offw,
        own_nodes=own_nodes, deg=deg,
    )
    return cores, consts


def _fold_weights(Wk, Wq, Wv, Wa, rel_att, rel_msg, rel_pri):
    Wk = np.asarray(Wk, np.float64)
    Wq = np.asarray(Wq, np.float64)
    Wv = np.asarray(Wv, np.float64)
    Wa = np.asarray(Wa, np.float64)
    rel_att = np.asarray(rel_att, np.float64)
    rel_msg = np.asarray(rel_msg, np.float64)
    rel_pri = np.asarray(rel_pri, np.float64)
    sd = float(np.sqrt(np.float32(HS)))

    wkm = np.zeros((IN, NT * ET, 2, D), np.float64)
    for nt in range(NT):
        for et in range(ET):
            Batt = np.zeros((D, D))
            Bmsg = np.zeros((D, D))
            for h in range(H):
                A = rel_att[h, et]
                Batt[h * HS : (h + 1) * HS, h * HS : (h + 1) * HS] = (
                    A @ A.T * rel_pri[h, et] / sd
                )
                Bmsg[h * HS : (h + 1) * HS, h * HS : (h + 1) * HS] = rel_msg[h, et]
            g = nt * ET + et
            wkm[:, g, 0] = Wk[nt] @ Batt
            wkm[:, g, 1] = Wv[nt] @ Bmsg
    import ml_dtypes

    wkm = wkm.reshape(IN, NT * ET * 2 * D).astype(ml_dtypes.bfloat16)
    wq4 = np.concatenate([Wq[t] for t in range(NT)], axis=0).astype(np.float32)
    wa_all = np.concatenate([Wa[t] for t in range(NT)], axis=1).astype(np.float32)
    return wkm, wq4, wa_all


def _build_program(consts):
    import concourse.mybir as mybir
    import concourse.tile as tile
    from concourse import bacc
    from concourse.masks import make_identity

    f32 = mybir.dt.float32
    bf16 = mybir.dt.bfloat16
    i16 = mybir.dt.int16
    W = consts["W"]
    WMAX = consts["WMAX"]
    NCH, tile_chunk = consts["NCH"], consts["tile_chunk"]
    chunk_tiles = [
        [t for t in range(NTILES) if tile_chunk[t] == h] for h in range(NCH)
    ]
    R, gbase, CHRs, RB = consts["R"], consts["gbase"], consts["CHRs"], consts["RB"]
    RPtot, IDXW, ABW = consts["RPtot"], consts["IDXW"], consts["ABW"]
    off8, offw = consts["off8"], consts["offw"]

    nc = bacc.Bacc("TRN2", target_bir_lowering=False, debug=False, num_devices=C)

    xpT = nc.dram_tensor("xpT", [IN, RPtot], bf16, kind="ExternalInput").ap()
    wkm = nc.dram_tensor("wkm", [IN, NT * ET * 2 * D], bf16, kind="ExternalInput").ap()
    x4T = nc.dram_tensor("x4T", [NT * IN, NL], f32, kind="ExternalInput").ap()
    wq4 = nc.dram_tensor("wq4", [NT * IN, D], f32, kind="ExternalInput").ap()
    wa = nc.dram_tensor("wa", [D, NT * D], f32, kind="ExternalInput").ap()
    oneh = nc.dram_tensor("oneh", [NL, NT], f32, kind="ExternalInput").ap()
    kmidx = nc.dram_tensor("kmidx", [128, IDXW], i16, kind="ExternalInput").ap()
    abias = nc.dram_tensor("abias", [128, ABW], f32, kind="ExternalInput").ap()
    outp = nc.dram_tensor("outp", [NL, D], f32, kind="ExternalOutput").ap()
    kmtab = [
        nc.dram_tensor(f"kmtab{h}", [CHRs[h], 2 * D], f32, kind="Internal").ap()
        for h in range(NCH)
    ]

    with tile.TileContext(nc) as tc:
        with tc.tile_pool(name="const", bufs=1) as constp, \
             tc.tile_pool(name="stage", bufs=2) as stage, \
             tc.tile_pool(name="work", bufs=3) as work, \
             tc.tile_pool(name="npsum", bufs=2, space="PSUM") as npsum, \
             tc.tile_pool(name="qpsum", bufs=2, space="PSUM") as qpsum, \
             tc.tile_pool(name="opsum", bufs=2, space="PSUM") as opsum:

            # ---- persistent constants ----
            # only wkm is needed for the chunk-0 table build; the rest load
            # via late_consts() after chunk 0's slabs are queued, so the first
            # table build (which gates the first gather) hits DMA sooner.
            wkm_s = constp.tile([IN, NT * ET * 2 * D], bf16, name="wkm_s", tag="wkm_s")
            nc.sync.dma_start(out=wkm_s[:], in_=wkm[:, :])
            wq4_s = constp.tile([128, 2 * D], f32, name="wq4_s", tag="wq4_s")
            wa_s = constp.tile([D, NT * D], f32, name="wa_s", tag="wa_s")
            oneh_s = constp.tile([128, NTILES * NT], f32, name="oneh_s", tag="oneh_s")
            kmidx_s = constp.tile([128, IDXW], i16, name="kmidx_s", tag="kmidx_s")
            abias_s = constp.tile([128, ABW], f32, name="abias_s", tag="abias_s")
            ident = constp.tile([128, 128], f32, name="ident", tag="ident")
            qall = constp.tile([128, NTILES * D], f32, name="qall", tag="qall")

            def late_consts():
                for k in range(2):
                    nc.sync.dma_start(
                        out=wq4_s[:, k * D : (k + 1) * D],
                        in_=wq4[k * 128 : (k + 1) * 128, :],
                    )
                nc.sync.dma_start(out=wa_s[:], in_=wa[:, :])
                nc.sync.dma_start(
                    out=oneh_s[:].rearrange("p (t f) -> p t f", t=NTILES),
                    in_=oneh[:, :].rearrange("(t p) f -> p t f", p=128),
                )
                nc.sync.dma_start(out=kmidx_s[:], in_=kmidx[:, :])
                nc.sync.dma_start(out=abias_s[:], in_=abias[:, :])
                make_identity(nc, ident[:])

            # ---- Q phase (emitted per chunk, 4 node-tiles per load) ----
            def q_tiles(ts):
                for g0 in range(0, len(ts), 4):
                    grp = ts[g0 : g0 + 4]
                    t0, ng = grp[0], len(grp)
                    x4_s = stage.tile([128, 2, 512], f32, name=f"x4_{t0}", tag="x4")
                    nc.sync.dma_start(
                        out=x4_s[:, :, : ng * 128],
                        in_=x4T[:, t0 * 128 : (t0 + ng) * 128].rearrange(
                            "(k p) n -> p k n", p=128
                        ),
                    )
                    for i in range(ng):
                        t = t0 + i
                        q_p = qpsum.tile(
                            [128, D], f32, space="PSUM", name=f"q_p{t}", tag="q_p"
                        )
                        for k in range(2):
                            nc.tensor.matmul(
                                q_p[:],
                                lhsT=x4_s[:, k, i * 128 : (i + 1) * 128],
                                rhs=wq4_s[:, k * D : (k + 1) * D],
                                start=(k == 0),
                                stop=(k == 1),
                            )
                        nc.any.tensor_copy(out=qall[:, t * D : (t + 1) * D], in_=q_p[:])

            # ---- node/pair-table phase per chunk ----
            # Groups are packed back-to-back (no 128-row alignment); each
            # 128-row output tile may span several (ntype, etype) groups, so
            # it gets one partial-M matmul per overlapped group.
            def node_chunk_emitters(h):
                bounds = []  # (start_row, end_row, g) for nonempty groups
                for g in range(NT * ET):
                    if int(R[h, g]) > 0:
                        bounds.append((int(gbase[h, g]), int(gbase[h, g]) + int(R[h, g]), g))
                GT = bounds[-1][1]  # real rows (64-aligned)
                n_tiles = (GT + 127) // 128
                SLAB = 16
                emitters = []
                for s0 in range(0, n_tiles, SLAB):
                    emitters.append(
                        lambda s0=s0: node_slab(h, bounds, GT, n_tiles, SLAB, s0)
                    )
                return emitters

            def node_slab(h, bounds, GT, n_tiles, SLAB, s0):
                if True:
                    nb = min(SLAB, n_tiles - s0)
                    row0 = s0 * 128
                    rows = min(GT, (s0 + nb) * 128) - row0
                    lhs_s = stage.tile(
                        [IN, SLAB * 128], bf16, name=f"lhs_{h}_{s0}", tag="lhs"
                    )
                    nc.sync.dma_start(
                        out=lhs_s[:, :rows],
                        in_=xpT[:, int(RB[h]) + row0 : int(RB[h]) + row0 + rows],
                    )
                    slab = stage.tile(
                        [128, SLAB, 2 * D], f32, name=f"slab_{h}_{s0}", tag="slab"
                    )
                    for i in range(0, nb, 4):
                        nn = min(4, nb - i)
                        km_p = npsum.tile(
                            [128, 512], f32, space="PSUM", name=f"km_p{h}_{s0}_{i}", tag="km_p"
                        )
                        covers = []
                        for j in range(nn):
                            t0 = row0 + (i + j) * 128  # tile's first table row
                            covers.append(min(128, GT - t0))
                            for gs, ge, g in bounds:
                                lo, hi = max(gs, t0), min(ge, t0 + 128)
                                if lo >= hi:
                                    continue
                                nc.tensor.matmul(
                                    km_p[lo - t0 : hi - t0, j * 128 : (j + 1) * 128],
                                    lhsT=lhs_s[:, (i + j) * 128 + lo - t0 : (i + j) * 128 + hi - t0],
                                    rhs=wkm_s[:, g * 128 : (g + 1) * 128],
                                    start=True,
                                    stop=True,
                                )
                        if covers[-1] == 128:
                            nc.any.tensor_copy(
                                out=slab[:, i : i + nn],
                                in_=km_p[:].rearrange("p (a d) -> p a d", a=4)[:, :nn],
                            )
                        else:
                            for j in range(nn):
                                nc.any.tensor_copy(
                                    out=slab[: covers[j], i + j],
                                    in_=km_p[: covers[j], j * 128 : (j + 1) * 128],
                                )
                    nf = rows // 128  # full tiles in this slab
                    if nf:
                        nc.sync.dma_start(
                            out=kmtab[h][row0 : row0 + nf * 128, :].rearrange(
                                "(a p) d -> p a d", p=128
                            ),
                            in_=slab[:, :nf],
                        )
                    if rows % 128:
                        pr = rows % 128
                        nc.sync.dma_start(
                            out=kmtab[h][row0 + nf * 128 : row0 + rows, :],
                            in_=slab[:pr, nf],
                        )

            # ---- phase 3: per node-tile softmax + aggregation ----
            def p3_tile(t):
                h = int(tile_chunk[t])
                wt = int(W[t])
                n_idx = 128 * wt
                o8, ow = int(off8[t]), int(offw[t])
                gt = work.tile([128, WMAX, 2 * D], f32, name=f"gt{t}", tag="gt")
                nc.gpsimd.dma_gather(
                    out_ap=gt[:, :wt],
                    in_ap=kmtab[h][:, :],
                    idxs_ap=kmidx_s[:, o8 : o8 + 8 * wt],
                    num_idxs=n_idx,
                    num_idxs_reg=n_idx,
                    elem_size=2 * D,
                    single_packet=False,
                )
                aprod = work.tile([128, WMAX, D], f32, name=f"aprod{t}", tag="aprod")
                qb = qall[:, t * D : (t + 1) * D].unsqueeze(1).to_broadcast([128, wt, D])
                nc.vector.tensor_tensor(
                    out=aprod[:, :wt], in0=gt[:, :wt, :D], in1=qb, op=mybir.AluOpType.mult
                )
                am = work.tile([128, H, WMAX], f32, name=f"am{t}", tag="am")
                nc.vector.tensor_reduce(
                    out=am[:, :, :wt],
                    in_=aprod[:, :wt].rearrange("p w (h d) -> p h w d", h=H),
                    axis=mybir.AxisListType.X,
                    op=mybir.AluOpType.add,
                )
                amb = work.tile([128, H, WMAX], f32, name=f"amb{t}", tag="amb")
                bb = abias_s[:, ow : ow + wt].unsqueeze(1).to_broadcast([128, H, wt])
                nc.vector.tensor_tensor(
                    out=amb[:, :, :wt], in0=am[:, :, :wt], in1=bb, op=mybir.AluOpType.add
                )
                # softmax without max-subtraction: |a| is bounded well below
                # f32 exp overflow, and pads carry a -1e30 bias -> exp == 0.
                ex = work.tile([128, H, WMAX], f32, name=f"ex{t}", tag="ex")
                nc.scalar.activation(
                    out=ex[:, :, :wt], in_=amb[:, :, :wt],
                    func=mybir.ActivationFunctionType.Exp,
                )
                den = work.tile([128, H], f32, name=f"den{t}", tag="den")
                nc.vector.tensor_reduce(
                    out=den[:], in_=ex[:, :, :wt],
                    axis=mybir.AxisListType.X, op=mybir.AluOpType.add,
                )
                rden = work.tile([128, H], f32, name=f"rden{t}", tag="rden")
                nc.vector.reciprocal(out=rden[:], in_=den[:])
                mprod = work.tile([128, H, HS, WMAX], f32, name=f"mprod{t}", tag="mprod")
                mpart = gt[:, :wt, D:].rearrange("p w (h d) -> p h d w", h=H)
                ab2 = ex[:, :, :wt].unsqueeze(2).to_broadcast([128, H, HS, wt])
                nc.vector.tensor_tensor(
                    out=mprod[:, :, :, :wt], in0=mpart, in1=ab2, op=mybir.AluOpType.mult
                )
                hm = work.tile([128, D], f32, name=f"hm{t}", tag="hm")
                nc.vector.tensor_reduce(
                    out=hm[:].rearrange("p (h d) -> p h d", h=H),
                    in_=mprod[:, :, :, :wt],
                    axis=mybir.AxisListType.X,
                    op=mybir.AluOpType.add,
                )
                hm2 = work.tile([128, D], f32, name=f"hm2{t}", tag="hm2")
                nc.vector.tensor_tensor(
                    out=hm2[:].rearrange("p (h d) -> p h d", h=H),
                    in0=hm[:].rearrange("p (h d) -> p h d", h=H),
                    in1=rden[:].unsqueeze(2).to_broadcast([128, H, HS]),
                    op=mybir.AluOpType.mult,
                )
                tp = opsum.tile([128, 128], f32, space="PSUM", name=f"tp{t}", tag="tp")
                nc.tensor.transpose(out=tp[:D, :], in_=hm2[:], identity=ident[:])
                hT = work.tile([D, 128], f32, name=f"hT{t}", tag="hT")
                nc.any.tensor_copy(out=hT[:], in_=tp[:D, :])
                o4 = opsum.tile([128, NT * D], f32, space="PSUM", name=f"o4_{t}", tag="o4")
                nc.tensor.matmul(o4[:], lhsT=hT[:], rhs=wa_s[:], start=True, stop=True)
                osel = work.tile([128, NT * D], f32, name=f"osel{t}", tag="osel")
                ohb = (
                    oneh_s[:]
                    .rearrange("p (t f) -> p t f", t=NTILES)[:, t]
                    .unsqueeze(1)
                    .to_broadcast([128, D, NT])
                )
                nc.vector.tensor_tensor(
                    out=osel[:].rearrange("p (t d) -> p d t", t=NT),
                    in0=o4[:].rearrange("p (t d) -> p d t", t=NT),
                    in1=ohb,
                    op=mybir.AluOpType.mult,
                )
                ot = work.tile([128, D], f32, name=f"ot{t}", tag="ot")
                nc.vector.tensor_reduce(
                    out=ot[:],
                    in_=osel[:].rearrange("p (t d) -> p d t", t=NT),
                    axis=mybir.AxisListType.X,
                    op=mybir.AluOpType.add,
                )
                nc.sync.dma_start(out=outp[t * 128 : (t + 1) * 128, :], in_=ot[:])

            # emission order = scheduler priority: build chunk 0's table first,
            # then interleave later chunks' table slabs with phase 3 of the
            # already-built chunks so DMA/PE/DVE overlap across phases.
            import os

            mode = os.environ.get("GNN_EMIT", "chunk")
            if mode == "chunk":
                for em in node_chunk_emitters(0):
                    em()
                late_consts()
                q_tiles(chunk_tiles[0])
                for h in range(1, NCH):
                    for em in node_chunk_emitters(h):
                        em()
                    q_tiles(chunk_tiles[h])
                    for t in chunk_tiles[h - 1]:
                        p3_tile(t)
                for t in chunk_tiles[NCH - 1]:
                    p3_tile(t)
            else:  # interleave
                for em in node_chunk_emitters(0):
                    em()
                late_consts()
                q_tiles(chunk_tiles[0])
                for h in range(1, NCH):
                    q_tiles(chunk_tiles[h])
                    slabs = node_chunk_emitters(h)
                    tiles = chunk_tiles[h - 1]
                    ns, ntl = len(slabs), len(tiles)
                    si = ti = 0
                    while si < ns or ti < ntl:
                        take = (si + 1) * ntl <= (ti + 1) * ns
                        if si < ns and (take or ti >= ntl):
                            slabs[si]()
                            si += 1
                        else:
                            p3_tile(tiles[ti])
                            ti += 1
                for t in chunk_tiles[NCH - 1]:
                    p3_tile(t)

    nc.compile()
    return nc


def kernel(x, ntype, etype, src, dst, Wk, Wq, Wv, Wa, rel_att, rel_msg, rel_pri):
    from concourse import bass_utils

    cores, consts = _host_prep(x, ntype, etype, src, dst)
    wkm, wq4, wa_all = _fold_weights(Wk, Wq, Wv, Wa, rel_att, rel_msg, rel_pri)

    struct_sig = (
        tuple(consts["W"].tolist()),
        consts["NCH"],
        tuple(consts["CHRs"]),
        tuple(consts["R"].ravel().tolist()),
    )
    if "prog" not in _cache or _cache["prog"][0] != struct_sig:
        _cache["prog"] = (struct_sig, _build_program(consts))
    nc = _cache["prog"][1]

    in_maps = [
        dict(
            xpT=d["xpT"], wkm=wkm, x4T=d["x4T"], wq4=wq4, wa=wa_all,
            oneh=d["oneh"], kmidx=d["kmidx"], abias=d["abias"],
        )
        for d in cores
    ]
    res = bass_utils.run_bass_kernel_spmd(nc, in_maps, core_ids=list(range(C)))
    global LAST_RESULT
    LAST_RESULT = res

    out = np.zeros((N, D), dtype=np.float32)
    own = consts["own_nodes"]
    for c in range(C):
        oc = res.results[c]["outp"]
        m = own[c] >= 0
        out[own[c][m]] = oc[m]
    out[consts["deg"] == 0] = 0.0
    return out



# revision 42
# speedup vs baseline: 1.1453x; 1.0126x over previous
"""HGT-style heterogeneous graph message passing on 8 Trainium2 cores.

Strategy:
 - Host folds the per-(head, etype) relation transforms into per-(ntype, etype)
   64x128 weight matrices:  a_e = <k'_src, q_dst>  with
   k' = x @ Wk[nt] @ blockdiag_h(A A^T * pri / sqrt(d)),  m = x @ Wv[nt] @ blockdiag_h(M).
 - dst nodes are sharded across the 8 cores round-robin by degree rank, so all
   segment ops (softmax max/sum, weighted aggregation) become dense row
   reductions over degree-sorted [128, W_t] tiles.  No collectives.
 - Each core computes a deduplicated (src, etype) pair table [rows, 128] =
   [k' | m] on device (PE matmuls), then dma_gather's the rows of its slots.
"""

import sys

sys.path.insert(0, "/opt/trn_rl_repo")

import numpy as np

N, E = 40000, 640000
IN, H, HS = 64, 4, 16
NT, ET = 4, 8
D = H * HS  # 64
C = 8  # cores
NL = 5120  # padded local nodes per core
NTILES = NL // 128  # 40
NEG = -1.0e30

_cache = {}
LAST_RESULT = None  # BassKernelResults of the most recent run (for test harness)


def _host_prep(x, ntype, etype, src, dst):
    """Returns per-core input arrays + structural constants."""
    x = np.ascontiguousarray(np.asarray(x, dtype=np.float32))
    nt_ = np.asarray(ntype).astype(np.int64)
    et_ = np.asarray(etype).astype(np.int64)
    src = np.asarray(src).astype(np.int64)
    dst = np.asarray(dst).astype(np.int64)

    deg = np.bincount(dst, minlength=N)
    order = np.argsort(-deg, kind="stable")
    ranks = np.empty(N, dtype=np.int64)
    ranks[order] = np.arange(N)
    core_of_node = ranks % C
    local_of_node = ranks // C

    # tile widths (shared across cores): tile t covers global ranks [1024t, 1024(t+1))
    W = np.zeros(NTILES, dtype=np.int64)
    deg_by_rank = deg[order]
    for t in range(NTILES):
        lo, hi = t * 1024, min((t + 1) * 1024, N)
        W[t] = max(int(deg_by_rank[lo:hi].max()) if hi > lo else 1, 1)

    percore = []
    for c in range(C):
        ei = np.nonzero(core_of_node[dst] == c)[0]
        ld = local_of_node[dst[ei]]
        o = np.argsort(ld, kind="stable")
        percore.append((ei[o], ld[o]))

    # table chunks: tiny leading chunks so the first gather starts early,
    # big trailing chunks to limit per-group 64-row alignment padding.
    # Each chunk's padded pair count must stay < 32000 (int16 gather idx).
    patterns = [[5] * 8, [4] * 10, [2] * 20]
    for pat in patterns:
        assert sum(pat) == NTILES
        NCH = len(pat)
        tile_chunk = np.repeat(np.arange(NCH), pat)  # [NTILES] -> chunk id
        cnts = np.zeros((C, NCH, NT * ET), dtype=np.int64)
        pair_data = []
        for c in range(C):
            ei, ld = percore[c]
            ch_of = tile_chunk[ld // 128]
            key = src[ei] * ET + et_[ei]
            chunk_pairs = []
            for h in range(NCH):
                uk = np.unique(key[ch_of == h])  # sorted keys
                g = nt_[uk // ET] * ET + (uk % ET)
                np.add.at(cnts[c, h], g, 1)
                chunk_pairs.append((uk, g))
            pair_data.append(chunk_pairs)
        # 64-row group alignment: matmul output base partition must be 0/64
        R = 64 * ((cnts.max(axis=0) + 63) // 64)  # [NCH, 32]
        CHRs = 128 * ((R.sum(axis=1) + 127) // 128)
        if CHRs.max() < 32000:
            break
    else:
        raise RuntimeError("could not chunk tables under int16 limit")

    gbase = np.zeros((NCH, NT * ET), dtype=np.int64)
    for h in range(NCH):
        gbase[h] = np.concatenate(([0], np.cumsum(R[h])[:-1]))
    CHRs = [int(v) for v in CHRs]
    RB = np.concatenate(([0], np.cumsum(CHRs)[:-1])).astype(np.int64)
    RPtot = int(sum(CHRs))

    IDX8 = (8 * W).astype(np.int64)
    off8 = np.concatenate(([0], np.cumsum(IDX8)[:-1]))
    offw = np.concatenate(([0], np.cumsum(W)[:-1]))
    IDXW = int(IDX8.sum())
    ABW = int(W.sum())

    cores = []
    own_nodes = np.full((C, NL), -1, dtype=np.int64)
    for c in range(C):
        ei, ld = percore[c]
        etile = ld // 128
        ch_of = tile_chunk[etile]
        key = src[ei] * ET + et_[ei]

        ownc = order[c::C]
        own_nodes[c, : len(ownc)] = ownc

        rowid_of_edge = np.zeros(len(ei), dtype=np.int64)
        xp_node = np.full(RPtot, -1, dtype=np.int64)
        for h in range(NCH):
            uk, g = pair_data[c][h]  # uk sorted by key; g aligned
            po = np.argsort(g, kind="stable")
            gs = g[po]
            base_in_g = np.concatenate(
                ([0], np.cumsum(np.bincount(gs, minlength=NT * ET))[:-1])
            )
            rows_po = gbase[h][gs] + (np.arange(len(uk)) - base_in_g[gs])
            row_of_uk = np.empty(len(uk), dtype=np.int64)
            row_of_uk[po] = rows_po
            xp_node[RB[h] + row_of_uk] = uk // ET
            sel = np.nonzero(ch_of == h)[0]
            rowid_of_edge[sel] = row_of_uk[np.searchsorted(uk, key[sel])]

        import ml_dtypes

        xpT = np.zeros((IN, RPtot), dtype=ml_dtypes.bfloat16)
        valid = xp_node >= 0
        xpT[:, valid] = x[xp_node[valid]].T.astype(ml_dtypes.bfloat16)

        cnt = np.bincount(ld, minlength=NL)
        starts = np.concatenate(([0], np.cumsum(cnt)[:-1]))
        jpos = np.arange(len(ei)) - starts[ld]
        p_of = ld % 128

        kmidx = np.zeros((128, IDXW), dtype=np.int16)
        abias = np.full((128, ABW), NEG, dtype=np.float32)
        for t in range(NTILES):
            wt = int(W[t])
            sel = np.nonzero(etile == t)[0]
            M = np.zeros((128, wt), dtype=np.int16)
            M[p_of[sel], jpos[sel]] = rowid_of_edge[sel].astype(np.int16)
            idsl = M.T.ravel()  # list position k = j*128 + p
            wrapped = idsl.reshape(8 * wt, 16).T
            kmidx[:, int(off8[t]) : int(off8[t]) + 8 * wt] = np.tile(wrapped, (8, 1))
            B = np.full((128, wt), NEG, dtype=np.float32)
            B[p_of[sel], jpos[sel]] = 0.0
            abias[:, int(offw[t]) : int(offw[t]) + wt] = B

        x4T = np.zeros((NT * IN, NL), dtype=np.float32)
        nreal = len(ownc)
        ntc = nt_[ownc]
        xo = x[ownc]
        for t4 in range(NT):
            m4 = ntc == t4
            x4T[t4 * IN : (t4 + 1) * IN, :nreal][:, m4] = xo[m4].T

        oneh = np.zeros((NL, NT), dtype=np.float32)
        oneh[np.arange(nreal), ntc] = 1.0

        cores.append(dict(xpT=xpT, x4T=x4T, oneh=oneh, kmidx=kmidx, abias=abias))

    consts = dict(
        W=W, WMAX=int(W.max()), NCH=NCH, tile_chunk=tile_chunk, R=R, gbase=gbase,
        CHRs=CHRs, RB=RB, RPtot=RPtot, IDXW=IDXW, ABW=ABW, off8=off8, offw=offw,
        own_nodes=own_nodes, deg=deg,
    )
    return cores, consts


def _fold_weights(Wk, Wq, Wv, Wa, rel_att, rel_msg, rel_pri):
    Wk = np.asarray(Wk, np.float64)
    Wq = np.asarray(Wq, np.float64)
    Wv = np.asarray(Wv, np.float64)
    Wa = np.asarray(Wa, np.float64)
    rel_att = np.asarray(rel_att, np.float64)
    rel_msg = np.asarray(rel_msg, np.float64)
    rel_pri = np.asarray(rel_pri, np.float64)
    sd = float(np.sqrt(np.float32(HS)))

    wkm = np.zeros((IN, NT * ET, 2, D), np.float64)
    for nt in range(NT):
        for et in range(ET):
            Batt = np.zeros((D, D))
            Bmsg = np.zeros((D, D))
            for h in range(H):
                A = rel_att[h, et]
                Batt[h * HS : (h + 1) * HS, h * HS : (h + 1) * HS] = (
                    A @ A.T * rel_pri[h, et] / sd
                )
                Bmsg[h * HS : (h + 1) * HS, h * HS : (h + 1) * HS] = rel_msg[h, et]
            g = nt * ET + et
            wkm[:, g, 0] = Wk[nt] @ Batt
            wkm[:, g, 1] = Wv[nt] @ Bmsg
    import ml_dtypes

    wkm = wkm.reshape(IN, NT * ET * 2 * D).astype(ml_dtypes.bfloat16)
    wq4 = np.concatenate([Wq[t] for t in range(NT)], axis=0).astype(np.float32)
    wa_all = np.concatenate([Wa[t] for t in range(NT)], axis=1).astype(np.float32)
    return wkm, wq4, wa_all


def _build_program(consts):
    import concourse.mybir as mybir
    import concourse.tile as tile
    from concourse import bacc
    from concourse.masks import make_identity

    f32 = mybir.dt.float32
    bf16 = mybir.dt.bfloat16
    i16 = mybir.dt.int16
    W = consts["W"]
    WMAX = consts["WMAX"]
    NCH, tile_chunk = consts["NCH"], consts["tile_chunk"]
    chunk_tiles = [
        [t for t in range(NTILES) if tile_chunk[t] == h] for h in range(NCH)
    ]
    R, gbase, CHRs, RB = consts["R"], consts["gbase"], consts["CHRs"], consts["RB"]
    RPtot, IDXW, ABW = consts["RPtot"], consts["IDXW"], consts["ABW"]
    off8, offw = consts["off8"], consts["offw"]

    nc = bacc.Bacc("TRN2", target_bir_lowering=False, debug=False, num_devices=C)

    xpT = nc.dram_tensor("xpT", [IN, RPtot], bf16, kind="ExternalInput").ap()
    wkm = nc.dram_tensor("wkm", [IN, NT * ET * 2 * D], bf16, kind="ExternalInput").ap()
    x4T = nc.dram_tensor("x4T", [NT * IN, NL], f32, kind="ExternalInput").ap()
    wq4 = nc.dram_tensor("wq4", [NT * IN, D], f32, kind="ExternalInput").ap()
    wa = nc.dram_tensor("wa", [D, NT * D], f32, kind="ExternalInput").ap()
    oneh = nc.dram_tensor("oneh", [NL, NT], f32, kind="ExternalInput").ap()
    kmidx = nc.dram_tensor("kmidx", [128, IDXW], i16, kind="ExternalInput").ap()
    abias = nc.dram_tensor("abias", [128, ABW], f32, kind="ExternalInput").ap()
    outp = nc.dram_tensor("outp", [NL, D], f32, kind="ExternalOutput").ap()
    kmtab = [
        nc.dram_tensor(f"kmtab{h}", [CHRs[h], 2 * D], bf16, kind="Internal").ap()
        for h in range(NCH)
    ]

    with tile.TileContext(nc) as tc:
        with tc.tile_pool(name="const", bufs=1) as constp, \
             tc.tile_pool(name="stage", bufs=2) as stage, \
             tc.tile_pool(name="work", bufs=3) as work, \
             tc.tile_pool(name="npsum", bufs=2, space="PSUM") as npsum, \
             tc.tile_pool(name="qpsum", bufs=2, space="PSUM") as qpsum, \
             tc.tile_pool(name="opsum", bufs=2, space="PSUM") as opsum:

            # ---- persistent constants ----
            # only wkm is needed for the chunk-0 table build; the rest load
            # via late_consts() after chunk 0's slabs are queued, so the first
            # table build (which gates the first gather) hits DMA sooner.
            wkm_s = constp.tile([IN, NT * ET * 2 * D], bf16, name="wkm_s", tag="wkm_s")
            nc.sync.dma_start(out=wkm_s[:], in_=wkm[:, :])
            wq4_s = constp.tile([128, 2 * D], f32, name="wq4_s", tag="wq4_s")
            wa_s = constp.tile([D, NT * D], f32, name="wa_s", tag="wa_s")
            oneh_s = constp.tile([128, NTILES * NT], f32, name="oneh_s", tag="oneh_s")
            kmidx_s = constp.tile([128, IDXW], i16, name="kmidx_s", tag="kmidx_s")
            abias_s = constp.tile([128, ABW], f32, name="abias_s", tag="abias_s")
            ident = constp.tile([128, 128], f32, name="ident", tag="ident")
            qall = constp.tile([128, NTILES * D], bf16, name="qall", tag="qall")

            def late_consts():
                for k in range(2):
                    nc.sync.dma_start(
                        out=wq4_s[:, k * D : (k + 1) * D],
                        in_=wq4[k * 128 : (k + 1) * 128, :],
                    )
                nc.sync.dma_start(out=wa_s[:], in_=wa[:, :])
                nc.sync.dma_start(
                    out=oneh_s[:].rearrange("p (t f) -> p t f", t=NTILES),
                    in_=oneh[:, :].rearrange("(t p) f -> p t f", p=128),
                )
                nc.sync.dma_start(out=kmidx_s[:], in_=kmidx[:, :])
                nc.sync.dma_start(out=abias_s[:], in_=abias[:, :])
                make_identity(nc, ident[:])

            # ---- Q phase (emitted per chunk, 4 node-tiles per load) ----
            def q_tiles(ts):
                for g0 in range(0, len(ts), 4):
                    grp = ts[g0 : g0 + 4]
                    t0, ng = grp[0], len(grp)
                    x4_s = stage.tile([128, 2, 512], f32, name=f"x4_{t0}", tag="x4")
                    nc.sync.dma_start(
                        out=x4_s[:, :, : ng * 128],
                        in_=x4T[:, t0 * 128 : (t0 + ng) * 128].rearrange(
                            "(k p) n -> p k n", p=128
                        ),
                    )
                    for i in range(ng):
                        t = t0 + i
                        q_p = qpsum.tile(
                            [128, D], f32, space="PSUM", name=f"q_p{t}", tag="q_p"
                        )
                        for k in range(2):
                            nc.tensor.matmul(
                                q_p[:],
                                lhsT=x4_s[:, k, i * 128 : (i + 1) * 128],
                                rhs=wq4_s[:, k * D : (k + 1) * D],
                                start=(k == 0),
                                stop=(k == 1),
                            )
                        nc.any.tensor_copy(out=qall[:, t * D : (t + 1) * D], in_=q_p[:])

            # ---- node/pair-table phase per chunk ----
            # Groups are packed back-to-back (no 128-row alignment); each
            # 128-row output tile may span several (ntype, etype) groups, so
            # it gets one partial-M matmul per overlapped group.
            def node_chunk_emitters(h):
                bounds = []  # (start_row, end_row, g) for nonempty groups
                for g in range(NT * ET):
                    if int(R[h, g]) > 0:
                        bounds.append((int(gbase[h, g]), int(gbase[h, g]) + int(R[h, g]), g))
                GT = bounds[-1][1]  # real rows (64-aligned)
                n_tiles = (GT + 127) // 128
                SLAB = 16
                emitters = []
                for s0 in range(0, n_tiles, SLAB):
                    emitters.append(
                        lambda s0=s0: node_slab(h, bounds, GT, n_tiles, SLAB, s0)
                    )
                return emitters

            def node_slab(h, bounds, GT, n_tiles, SLAB, s0):
                if True:
                    nb = min(SLAB, n_tiles - s0)
                    row0 = s0 * 128
                    rows = min(GT, (s0 + nb) * 128) - row0
                    lhs_s = stage.tile(
                        [IN, SLAB * 128], bf16, name=f"lhs_{h}_{s0}", tag="lhs"
                    )
                    nc.sync.dma_start(
                        out=lhs_s[:, :rows],
                        in_=xpT[:, int(RB[h]) + row0 : int(RB[h]) + row0 + rows],
                    )
                    slab = stage.tile(
                        [128, SLAB, 2 * D], bf16, name=f"slab_{h}_{s0}", tag="slab"
                    )
                    for i in range(0, nb, 4):
                        nn = min(4, nb - i)
                        km_p = npsum.tile(
                            [128, 512], f32, space="PSUM", name=f"km_p{h}_{s0}_{i}", tag="km_p"
                        )
                        covers = []
                        for j in range(nn):
                            t0 = row0 + (i + j) * 128  # tile's first table row
                            covers.append(min(128, GT - t0))
                            for gs, ge, g in bounds:
                                lo, hi = max(gs, t0), min(ge, t0 + 128)
                                if lo >= hi:
                                    continue
                                nc.tensor.matmul(
                                    km_p[lo - t0 : hi - t0, j * 128 : (j + 1) * 128],
                                    lhsT=lhs_s[:, (i + j) * 128 + lo - t0 : (i + j) * 128 + hi - t0],
                                    rhs=wkm_s[:, g * 128 : (g + 1) * 128],
                                    start=True,
                                    stop=True,
                                )
                        if covers[-1] == 128:
                            nc.any.tensor_copy(
                                out=slab[:, i : i + nn],
                                in_=km_p[:].rearrange("p (a d) -> p a d", a=4)[:, :nn],
                            )
                        else:
                            for j in range(nn):
                                nc.any.tensor_copy(
                                    out=slab[: covers[j], i + j],
                                    in_=km_p[: covers[j], j * 128 : (j + 1) * 128],
                                )
                    nf = rows // 128  # full tiles in this slab
                    if nf:
                        nc.sync.dma_start(
                            out=kmtab[h][row0 : row0 + nf * 128, :].rearrange(
                                "(a p) d -> p a d", p=128
                            ),
                            in_=slab[:, :nf],
                        )
                    if rows % 128:
                        pr = rows % 128
                        nc.sync.dma_start(
                            out=kmtab[h][row0 + nf * 128 : row0 + rows, :],
                            in_=slab[:pr, nf],
                        )

            # ---- phase 3: per node-tile softmax + aggregation ----
            def p3_tile(t):
                h = int(tile_chunk[t])
                wt = int(W[t])
                n_idx = 128 * wt
                o8, ow = int(off8[t]), int(offw[t])
                gt = work.tile([128, WMAX, 2 * D], bf16, name=f"gt{t}", tag="gt")
                nc.gpsimd.dma_gather(
                    out_ap=gt[:, :wt],
                    in_ap=kmtab[h][:, :],
                    idxs_ap=kmidx_s[:, o8 : o8 + 8 * wt],
                    num_idxs=n_idx,
                    num_idxs_reg=n_idx,
                    elem_size=2 * D,
                    single_packet=False,
                )
                aprod = work.tile([128, WMAX, D], f32, name=f"aprod{t}", tag="aprod")
                qb = qall[:, t * D : (t + 1) * D].unsqueeze(1).to_broadcast([128, wt, D])
                nc.vector.tensor_tensor(
                    out=aprod[:, :wt], in0=gt[:, :wt, :D], in1=qb, op=mybir.AluOpType.mult
                )
                am = work.tile([128, H, WMAX], f32, name=f"am{t}", tag="am")
                nc.vector.tensor_reduce(
                    out=am[:, :, :wt],
                    in_=aprod[:, :wt].rearrange("p w (h d) -> p h w d", h=H),
                    axis=mybir.AxisListType.X,
                    op=mybir.AluOpType.add,
                )
                amb = work.tile([128, H, WMAX], f32, name=f"amb{t}", tag="amb")
                bb = abias_s[:, ow : ow + wt].unsqueeze(1).to_broadcast([128, H, wt])
                nc.vector.tensor_tensor(
                    out=amb[:, :, :wt], in0=am[:, :, :wt], in1=bb, op=mybir.AluOpType.add
                )
                # softmax without max-subtraction: |a| is bounded well below
                # f32 exp overflow, and pads carry a -1e30 bias -> exp == 0.
                ex = work.tile([128, H, WMAX], bf16, name=f"ex{t}", tag="ex")
                nc.scalar.activation(
                    out=ex[:, :, :wt], in_=amb[:, :, :wt],
                    func=mybir.ActivationFunctionType.Exp,
                )
                den = work.tile([128, H], f32, name=f"den{t}", tag="den")
                nc.vector.tensor_reduce(
                    out=den[:], in_=ex[:, :, :wt],
                    axis=mybir.AxisListType.X, op=mybir.AluOpType.add,
                )
                rden = work.tile([128, H], f32, name=f"rden{t}", tag="rden")
                nc.vector.reciprocal(out=rden[:], in_=den[:])
                mprod = work.tile([128, H, HS, WMAX], f32, name=f"mprod{t}", tag="mprod")
                mpart = gt[:, :wt, D:].rearrange("p w (h d) -> p h d w", h=H)
                ab2 = ex[:, :, :wt].unsqueeze(2).to_broadcast([128, H, HS, wt])
                nc.vector.tensor_tensor(
                    out=mprod[:, :, :, :wt], in0=mpart, in1=ab2, op=mybir.AluOpType.mult
                )
                hm = work.tile([128, D], f32, name=f"hm{t}", tag="hm")
                nc.vector.tensor_reduce(
                    out=hm[:].rearrange("p (h d) -> p h d", h=H),
                    in_=mprod[:, :, :, :wt],
                    axis=mybir.AxisListType.X,
                    op=mybir.AluOpType.add,
                )
                hm2 = work.tile([128, D], f32, name=f"hm2{t}", tag="hm2")
                nc.vector.tensor_tensor(
                    out=hm2[:].rearrange("p (h d) -> p h d", h=H),
                    in0=hm[:].rearrange("p (h d) -> p h d", h=H),
                    in1=rden[:].unsqueeze(2).to_broadcast([128, H, HS]),
                    op=mybir.AluOpType.mult,
                )
                tp = opsum.tile([128, 128], f32, space="PSUM", name=f"tp{t}", tag="tp")
                nc.tensor.transpose(out=tp[:D, :], in_=hm2[:], identity=ident[:])
                hT = work.tile([D, 128], f32, name=f"hT{t}", tag="hT")
                nc.any.tensor_copy(out=hT[:], in_=tp[:D, :])
                o4 = opsum.tile([128, NT * D], f32, space="PSUM", name=f"o4_{t}", tag="o4")
                nc.tensor.matmul(o4[:], lhsT=hT[:], rhs=wa_s[:], start=True, stop=True)
                osel = work.tile([128, NT * D], f32, name=f"osel{t}", tag="osel")
                ohb = (
                    oneh_s[:]
                    .rearrange("p (t f) -> p t f", t=NTILES)[:, t]
                    .unsqueeze(1)
                    .to_broadcast([128, D, NT])
                )
                nc.vector.tensor_tensor(
                    out=osel[:].rearrange("p (t d) -> p d t", t=NT),
                    in0=o4[:].rearrange("p (t d) -> p d t", t=NT),
                    in1=ohb,
                    op=mybir.AluOpType.mult,
                )
                ot = work.tile([128, D], f32, name=f"ot{t}", tag="ot")
                nc.vector.tensor_reduce(
                    out=ot[:],
                    in_=osel[:].rearrange("p (t d) -> p d t", t=NT),
                    axis=mybir.AxisListType.X,
                    op=mybir.AluOpType.add,
                )
                nc.sync.dma_start(out=outp[t * 128 : (t + 1) * 128, :], in_=ot[:])

            # emission order = scheduler priority: build chunk 0's table first,
            # then interleave later chunks' table slabs with phase 3 of the
            # already-built chunks so DMA/PE/DVE overlap across phases.
            import os

            mode = os.environ.get("GNN_EMIT", "chunk")
            if mode == "chunk":
                for em in node_chunk_emitters(0):
                    em()
                late_consts()
                q_tiles(chunk_tiles[0])
                for h in range(1, NCH):
                    for em in node_chunk_emitters(h):
                        em()
                    q_tiles(chunk_tiles[h])
                    for t in chunk_tiles[h - 1]:
                        p3_tile(t)
                for t in chunk_tiles[NCH - 1]:
                    p3_tile(t)
            else:  # interleave
                for em in node_chunk_emitters(0):
                    em()
                late_consts()
                q_tiles(chunk_tiles[0])
                for h in range(1, NCH):
                    q_tiles(chunk_tiles[h])
                    slabs = node_chunk_emitters(h)
                    tiles = chunk_tiles[h - 1]
                    ns, ntl = len(slabs), len(tiles)
                    si = ti = 0
                    while si < ns or ti < ntl:
                        take = (si + 1) * ntl <= (ti + 1) * ns
                        if si < ns and (take or ti >= ntl):
                            slabs[si]()
                            si += 1
                        else:
                            p3_tile(tiles[ti])
                            ti += 1
                for t in chunk_tiles[NCH - 1]:
                    p3_tile(t)

    nc.compile()
    return nc


def kernel(x, ntype, etype, src, dst, Wk, Wq, Wv, Wa, rel_att, rel_msg, rel_pri):
    from concourse import bass_utils

    cores, consts = _host_prep(x, ntype, etype, src, dst)
    wkm, wq4, wa_all = _fold_weights(Wk, Wq, Wv, Wa, rel_att, rel_msg, rel_pri)

    struct_sig = (
        tuple(consts["W"].tolist()),
        consts["NCH"],
        tuple(consts["CHRs"]),
        tuple(consts["R"].ravel().tolist()),
    )
    if "prog" not in _cache or _cache["prog"][0] != struct_sig:
        _cache["prog"] = (struct_sig, _build_program(consts))
    nc = _cache["prog"][1]

    in_maps = [
        dict(
            xpT=d["xpT"], wkm=wkm, x4T=d["x4T"], wq4=wq4, wa=wa_all,
            oneh=d["oneh"], kmidx=d["kmidx"], abias=d["abias"],
        )
        for d in cores
    ]
    res = bass_utils.run_bass_kernel_spmd(nc, in_maps, core_ids=list(range(C)))
    global LAST_RESULT
    LAST_RESULT = res

    out = np.zeros((N, D), dtype=np.float32)
    own = consts["own_nodes"]
    for c in range(C):
        oc = res.results[c]["outp"]
        m = own[c] >= 0
        out[own[c][m]] = oc[m]
    out[consts["deg"] == 0] = 0.0
    return out



# revision 43
# speedup vs baseline: 1.1461x; 1.0007x over previous
"""HGT-style heterogeneous graph message passing on 8 Trainium2 cores.

Strategy:
 - Host folds the per-(head, etype) relation transforms into per-(ntype, etype)
   64x128 weight matrices:  a_e = <k'_src, q_dst>  with
   k' = x @ Wk[nt] @ blockdiag_h(A A^T * pri / sqrt(d)),  m = x @ Wv[nt] @ blockdiag_h(M).
 - dst nodes are sharded across the 8 cores round-robin by degree rank, so all
   segment ops (softmax max/sum, weighted aggregation) become dense row
   reductions over degree-sorted [128, W_t] tiles.  No collectives.
 - Each core computes a deduplicated (src, etype) pair table [rows, 128] =
   [k' | m] on device (PE matmuls), then dma_gather's the rows of its slots.
"""

import sys

sys.path.insert(0, "/opt/trn_rl_repo")

import numpy as np

N, E = 40000, 640000
IN, H, HS = 64, 4, 16
NT, ET = 4, 8
D = H * HS  # 64
C = 8  # cores
NL = 5120  # padded local nodes per core
NTILES = NL // 128  # 40
NEG = -1.0e30

_cache = {}
LAST_RESULT = None  # BassKernelResults of the most recent run (for test harness)


def _host_prep(x, ntype, etype, src, dst):
    """Returns per-core input arrays + structural constants."""
    x = np.ascontiguousarray(np.asarray(x, dtype=np.float32))
    nt_ = np.asarray(ntype).astype(np.int64)
    et_ = np.asarray(etype).astype(np.int64)
    src = np.asarray(src).astype(np.int64)
    dst = np.asarray(dst).astype(np.int64)

    deg = np.bincount(dst, minlength=N)
    order = np.argsort(-deg, kind="stable")
    ranks = np.empty(N, dtype=np.int64)
    ranks[order] = np.arange(N)
    core_of_node = ranks % C
    local_of_node = ranks // C

    # tile widths (shared across cores): tile t covers global ranks [1024t, 1024(t+1))
    W = np.zeros(NTILES, dtype=np.int64)
    deg_by_rank = deg[order]
    for t in range(NTILES):
        lo, hi = t * 1024, min((t + 1) * 1024, N)
        W[t] = max(int(deg_by_rank[lo:hi].max()) if hi > lo else 1, 1)

    percore = []
    for c in range(C):
        ei = np.nonzero(core_of_node[dst] == c)[0]
        ld = local_of_node[dst[ei]]
        o = np.argsort(ld, kind="stable")
        percore.append((ei[o], ld[o]))

    # table chunks: tiny leading chunks so the first gather starts early,
    # big trailing chunks to limit per-group 64-row alignment padding.
    # Each chunk's padded pair count must stay < 32000 (int16 gather idx).
    patterns = [[5] * 8, [4] * 10, [2] * 20]
    for pat in patterns:
        assert sum(pat) == NTILES
        NCH = len(pat)
        tile_chunk = np.repeat(np.arange(NCH), pat)  # [NTILES] -> chunk id
        cnts = np.zeros((C, NCH, NT * ET), dtype=np.int64)
        pair_data = []
        for c in range(C):
            ei, ld = percore[c]
            ch_of = tile_chunk[ld // 128]
            key = src[ei] * ET + et_[ei]
            chunk_pairs = []
            for h in range(NCH):
                uk = np.unique(key[ch_of == h])  # sorted keys
                g = nt_[uk // ET] * ET + (uk % ET)
                np.add.at(cnts[c, h], g, 1)
                chunk_pairs.append((uk, g))
            pair_data.append(chunk_pairs)
        # 64-row group alignment: matmul output base partition must be 0/64
        R = 64 * ((cnts.max(axis=0) + 63) // 64)  # [NCH, 32]
        CHRs = 128 * ((R.sum(axis=1) + 127) // 128)
        if CHRs.max() < 32000:
            break
    else:
        raise RuntimeError("could not chunk tables under int16 limit")

    gbase = np.zeros((NCH, NT * ET), dtype=np.int64)
    for h in range(NCH):
        gbase[h] = np.concatenate(([0], np.cumsum(R[h])[:-1]))
    CHRs = [int(v) for v in CHRs]
    RB = np.concatenate(([0], np.cumsum(CHRs)[:-1])).astype(np.int64)
    RPtot = int(sum(CHRs))

    IDX8 = (8 * W).astype(np.int64)
    off8 = np.concatenate(([0], np.cumsum(IDX8)[:-1]))
    offw = np.concatenate(([0], np.cumsum(W)[:-1]))
    IDXW = int(IDX8.sum())
    ABW = int(W.sum())

    cores = []
    own_nodes = np.full((C, NL), -1, dtype=np.int64)
    for c in range(C):
        ei, ld = percore[c]
        etile = ld // 128
        ch_of = tile_chunk[etile]
        key = src[ei] * ET + et_[ei]

        ownc = order[c::C]
        own_nodes[c, : len(ownc)] = ownc

        rowid_of_edge = np.zeros(len(ei), dtype=np.int64)
        xp_node = np.full(RPtot, -1, dtype=np.int64)
        for h in range(NCH):
            uk, g = pair_data[c][h]  # uk sorted by key; g aligned
            po = np.argsort(g, kind="stable")
            gs = g[po]
            base_in_g = np.concatenate(
                ([0], np.cumsum(np.bincount(gs, minlength=NT * ET))[:-1])
            )
            rows_po = gbase[h][gs] + (np.arange(len(uk)) - base_in_g[gs])
            row_of_uk = np.empty(len(uk), dtype=np.int64)
            row_of_uk[po] = rows_po
            xp_node[RB[h] + row_of_uk] = uk // ET
            sel = np.nonzero(ch_of == h)[0]
            rowid_of_edge[sel] = row_of_uk[np.searchsorted(uk, key[sel])]

        import ml_dtypes

        xpT = np.zeros((IN, RPtot), dtype=ml_dtypes.bfloat16)
        valid = xp_node >= 0
        xpT[:, valid] = x[xp_node[valid]].T.astype(ml_dtypes.bfloat16)

        cnt = np.bincount(ld, minlength=NL)
        starts = np.concatenate(([0], np.cumsum(cnt)[:-1]))
        jpos = np.arange(len(ei)) - starts[ld]
        p_of = ld % 128

        kmidx = np.zeros((128, IDXW), dtype=np.int16)
        abias = np.full((128, ABW), NEG, dtype=np.float32)
        for t in range(NTILES):
            wt = int(W[t])
            sel = np.nonzero(etile == t)[0]
            M = np.zeros((128, wt), dtype=np.int16)
            M[p_of[sel], jpos[sel]] = rowid_of_edge[sel].astype(np.int16)
            idsl = M.T.ravel()  # list position k = j*128 + p
            wrapped = idsl.reshape(8 * wt, 16).T
            kmidx[:, int(off8[t]) : int(off8[t]) + 8 * wt] = np.tile(wrapped, (8, 1))
            B = np.full((128, wt), NEG, dtype=np.float32)
            B[p_of[sel], jpos[sel]] = 0.0
            abias[:, int(offw[t]) : int(offw[t]) + wt] = B

        x4T = np.zeros((NT * IN, NL), dtype=np.float32)
        nreal = len(ownc)
        ntc = nt_[ownc]
        xo = x[ownc]
        for t4 in range(NT):
            m4 = ntc == t4
            x4T[t4 * IN : (t4 + 1) * IN, :nreal][:, m4] = xo[m4].T

        oneh = np.zeros((NL, NT), dtype=np.float32)
        oneh[np.arange(nreal), ntc] = 1.0

        cores.append(dict(xpT=xpT, x4T=x4T, oneh=oneh, kmidx=kmidx, abias=abias))

    consts = dict(
        W=W, WMAX=int(W.max()), NCH=NCH, tile_chunk=tile_chunk, R=R, gbase=gbase,
        CHRs=CHRs, RB=RB, RPtot=RPtot, IDXW=IDXW, ABW=ABW, off8=off8, offw=offw,
        own_nodes=own_nodes, deg=deg,
    )
    return cores, consts


def _fold_weights(Wk, Wq, Wv, Wa, rel_att, rel_msg, rel_pri):
    Wk = np.asarray(Wk, np.float64)
    Wq = np.asarray(Wq, np.float64)
    Wv = np.asarray(Wv, np.float64)
    Wa = np.asarray(Wa, np.float64)
    rel_att = np.asarray(rel_att, np.float64)
    rel_msg = np.asarray(rel_msg, np.float64)
    rel_pri = np.asarray(rel_pri, np.float64)
    sd = float(np.sqrt(np.float32(HS)))

    wkm = np.zeros((IN, NT * ET, 2, D), np.float64)
    for nt in range(NT):
        for et in range(ET):
            Batt = np.zeros((D, D))
            Bmsg = np.zeros((D, D))
            for h in range(H):
                A = rel_att[h, et]
                Batt[h * HS : (h + 1) * HS, h * HS : (h + 1) * HS] = (
                    A @ A.T * rel_pri[h, et] / sd
                )
                Bmsg[h * HS : (h + 1) * HS, h * HS : (h + 1) * HS] = rel_msg[h, et]
            g = nt * ET + et
            wkm[:, g, 0] = Wk[nt] @ Batt
            wkm[:, g, 1] = Wv[nt] @ Bmsg
    import ml_dtypes

    wkm = wkm.reshape(IN, NT * ET * 2 * D).astype(ml_dtypes.bfloat16)
    wq4 = np.concatenate([Wq[t] for t in range(NT)], axis=0).astype(np.float32)
    wa_all = np.concatenate([Wa[t] for t in range(NT)], axis=1).astype(np.float32)
    return wkm, wq4, wa_all


def _build_program(consts):
    import concourse.mybir as mybir
    import concourse.tile as tile
    from concourse import bacc
    from concourse.masks import make_identity

    f32 = mybir.dt.float32
    bf16 = mybir.dt.bfloat16
    i16 = mybir.dt.int16
    W = consts["W"]
    WMAX = consts["WMAX"]
    NCH, tile_chunk = consts["NCH"], consts["tile_chunk"]
    chunk_tiles = [
        [t for t in range(NTILES) if tile_chunk[t] == h] for h in range(NCH)
    ]
    R, gbase, CHRs, RB = consts["R"], consts["gbase"], consts["CHRs"], consts["RB"]
    RPtot, IDXW, ABW = consts["RPtot"], consts["IDXW"], consts["ABW"]
    off8, offw = consts["off8"], consts["offw"]

    nc = bacc.Bacc("TRN2", target_bir_lowering=False, debug=False, num_devices=C)

    xpT = nc.dram_tensor("xpT", [IN, RPtot], bf16, kind="ExternalInput").ap()
    wkm = nc.dram_tensor("wkm", [IN, NT * ET * 2 * D], bf16, kind="ExternalInput").ap()
    x4T = nc.dram_tensor("x4T", [NT * IN, NL], f32, kind="ExternalInput").ap()
    wq4 = nc.dram_tensor("wq4", [NT * IN, D], f32, kind="ExternalInput").ap()
    wa = nc.dram_tensor("wa", [D, NT * D], f32, kind="ExternalInput").ap()
    oneh = nc.dram_tensor("oneh", [NL, NT], f32, kind="ExternalInput").ap()
    kmidx = nc.dram_tensor("kmidx", [128, IDXW], i16, kind="ExternalInput").ap()
    abias = nc.dram_tensor("abias", [128, ABW], f32, kind="ExternalInput").ap()
    outp = nc.dram_tensor("outp", [NL, D], f32, kind="ExternalOutput").ap()
    kmtab = [
        nc.dram_tensor(f"kmtab{h}", [CHRs[h], 2 * D], bf16, kind="Internal").ap()
        for h in range(NCH)
    ]

    with tile.TileContext(nc) as tc:
        with tc.tile_pool(name="const", bufs=1) as constp, \
             tc.tile_pool(name="stage", bufs=3) as stage, \
             tc.tile_pool(name="work", bufs=3) as work, \
             tc.tile_pool(name="npsum", bufs=2, space="PSUM") as npsum, \
             tc.tile_pool(name="qpsum", bufs=2, space="PSUM") as qpsum, \
             tc.tile_pool(name="opsum", bufs=2, space="PSUM") as opsum:

            # ---- persistent constants ----
            # only wkm is needed for the chunk-0 table build; the rest load
            # via late_consts() after chunk 0's slabs are queued, so the first
            # table build (which gates the first gather) hits DMA sooner.
            wkm_s = constp.tile([IN, NT * ET * 2 * D], bf16, name="wkm_s", tag="wkm_s")
            nc.sync.dma_start(out=wkm_s[:], in_=wkm[:, :])
            wq4_s = constp.tile([128, 2 * D], f32, name="wq4_s", tag="wq4_s")
            wa_s = constp.tile([D, NT * D], f32, name="wa_s", tag="wa_s")
            oneh_s = constp.tile([128, NTILES * NT], f32, name="oneh_s", tag="oneh_s")
            kmidx_s = constp.tile([128, IDXW], i16, name="kmidx_s", tag="kmidx_s")
            abias_s = constp.tile([128, ABW], f32, name="abias_s", tag="abias_s")
            ident = constp.tile([128, 128], f32, name="ident", tag="ident")
            qall = constp.tile([128, NTILES * D], bf16, name="qall", tag="qall")

            def late_consts():
                for k in range(2):
                    nc.sync.dma_start(
                        out=wq4_s[:, k * D : (k + 1) * D],
                        in_=wq4[k * 128 : (k + 1) * 128, :],
                    )
                nc.sync.dma_start(out=wa_s[:], in_=wa[:, :])
                nc.sync.dma_start(
                    out=oneh_s[:].rearrange("p (t f) -> p t f", t=NTILES),
                    in_=oneh[:, :].rearrange("(t p) f -> p t f", p=128),
                )
                nc.sync.dma_start(out=kmidx_s[:], in_=kmidx[:, :])
                nc.sync.dma_start(out=abias_s[:], in_=abias[:, :])
                make_identity(nc, ident[:])

            # ---- Q phase (emitted per chunk, 4 node-tiles per load) ----
            def q_tiles(ts):
                for g0 in range(0, len(ts), 4):
                    grp = ts[g0 : g0 + 4]
                    t0, ng = grp[0], len(grp)
                    x4_s = stage.tile([128, 2, 512], f32, name=f"x4_{t0}", tag="x4")
                    nc.sync.dma_start(
                        out=x4_s[:, :, : ng * 128],
                        in_=x4T[:, t0 * 128 : (t0 + ng) * 128].rearrange(
                            "(k p) n -> p k n", p=128
                        ),
                    )
                    for i in range(ng):
                        t = t0 + i
                        q_p = qpsum.tile(
                            [128, D], f32, space="PSUM", name=f"q_p{t}", tag="q_p"
                        )
                        for k in range(2):
                            nc.tensor.matmul(
                                q_p[:],
                                lhsT=x4_s[:, k, i * 128 : (i + 1) * 128],
                                rhs=wq4_s[:, k * D : (k + 1) * D],
                                start=(k == 0),
                                stop=(k == 1),
                            )
                        nc.any.tensor_copy(out=qall[:, t * D : (t + 1) * D], in_=q_p[:])

            # ---- node/pair-table phase per chunk ----
            # Groups are packed back-to-back (no 128-row alignment); each
            # 128-row output tile may span several (ntype, etype) groups, so
            # it gets one partial-M matmul per overlapped group.
            def node_chunk_emitters(h):
                bounds = []  # (start_row, end_row, g) for nonempty groups
                for g in range(NT * ET):
                    if int(R[h, g]) > 0:
                        bounds.append((int(gbase[h, g]), int(gbase[h, g]) + int(R[h, g]), g))
                GT = bounds[-1][1]  # real rows (64-aligned)
                n_tiles = (GT + 127) // 128
                SLAB = 16
                emitters = []
                for s0 in range(0, n_tiles, SLAB):
                    emitters.append(
                        lambda s0=s0: node_slab(h, bounds, GT, n_tiles, SLAB, s0)
                    )
                return emitters

            def node_slab(h, bounds, GT, n_tiles, SLAB, s0):
                if True:
                    nb = min(SLAB, n_tiles - s0)
                    row0 = s0 * 128
                    rows = min(GT, (s0 + nb) * 128) - row0
                    lhs_s = stage.tile(
                        [IN, SLAB * 128], bf16, name=f"lhs_{h}_{s0}", tag="lhs"
                    )
                    nc.sync.dma_start(
                        out=lhs_s[:, :rows],
                        in_=xpT[:, int(RB[h]) + row0 : int(RB[h]) + row0 + rows],
                    )
                    slab = stage.tile(
                        [128, SLAB, 2 * D], bf16, name=f"slab_{h}_{s0}", tag="slab"
                    )
                    for i in range(0, nb, 4):
                        nn = min(4, nb - i)
                        km_p = npsum.tile(
                            [128, 512], f32, space="PSUM", name=f"km_p{h}_{s0}_{i}", tag="km_p"
                        )
                        covers = []
                        for j in range(nn):
                            t0 = row0 + (i + j) * 128  # tile's first table row
                            covers.append(min(128, GT - t0))
                            for gs, ge, g in bounds:
                                lo, hi = max(gs, t0), min(ge, t0 + 128)
                                if lo >= hi:
                                    continue
                                nc.tensor.matmul(
                                    km_p[lo - t0 : hi - t0, j * 128 : (j + 1) * 128],
                                    lhsT=lhs_s[:, (i + j) * 128 + lo - t0 : (i + j) * 128 + hi - t0],
                                    rhs=wkm_s[:, g * 128 : (g + 1) * 128],
                                    start=True,
                                    stop=True,
                                )
                        if covers[-1] == 128:
                            nc.any.tensor_copy(
                                out=slab[:, i : i + nn],
                                in_=km_p[:].rearrange("p (a d) -> p a d", a=4)[:, :nn],
                            )
                        else:
                            for j in range(nn):
                                nc.any.tensor_copy(
                                    out=slab[: covers[j], i + j],
                                    in_=km_p[: covers[j], j * 128 : (j + 1) * 128],
                                )
                    nf = rows // 128  # full tiles in this slab
                    if nf:
                        nc.sync.dma_start(
                            out=kmtab[h][row0 : row0 + nf * 128, :].rearrange(
                                "(a p) d -> p a d", p=128
                            ),
                            in_=slab[:, :nf],
                        )
                    if rows % 128:
                        pr = rows % 128
                        nc.sync.dma_start(
                            out=kmtab[h][row0 + nf * 128 : row0 + rows, :],
                            in_=slab[:pr, nf],
                        )

            # ---- phase 3: per node-tile softmax + aggregation ----
            def p3_tile(t):
                h = int(tile_chunk[t])
                wt = int(W[t])
                n_idx = 128 * wt
                o8, ow = int(off8[t]), int(offw[t])
                gt = work.tile([128, WMAX, 2 * D], bf16, name=f"gt{t}", tag="gt")
                nc.gpsimd.dma_gather(
                    out_ap=gt[:, :wt],
                    in_ap=kmtab[h][:, :],
                    idxs_ap=kmidx_s[:, o8 : o8 + 8 * wt],
                    num_idxs=n_idx,
                    num_idxs_reg=n_idx,
                    elem_size=2 * D,
                    single_packet=False,
                )
                aprod = work.tile([128, WMAX, D], f32, name=f"aprod{t}", tag="aprod")
                qb = qall[:, t * D : (t + 1) * D].unsqueeze(1).to_broadcast([128, wt, D])
                nc.vector.tensor_tensor(
                    out=aprod[:, :wt], in0=gt[:, :wt, :D], in1=qb, op=mybir.AluOpType.mult
                )
                am = work.tile([128, H, WMAX], f32, name=f"am{t}", tag="am")
                nc.vector.tensor_reduce(
                    out=am[:, :, :wt],
                    in_=aprod[:, :wt].rearrange("p w (h d) -> p h w d", h=H),
                    axis=mybir.AxisListType.X,
                    op=mybir.AluOpType.add,
                )
                amb = work.tile([128, H, WMAX], f32, name=f"amb{t}", tag="amb")
                bb = abias_s[:, ow : ow + wt].unsqueeze(1).to_broadcast([128, H, wt])
                nc.vector.tensor_tensor(
                    out=amb[:, :, :wt], in0=am[:, :, :wt], in1=bb, op=mybir.AluOpType.add
                )
                # softmax without max-subtraction: |a| is bounded well below
                # f32 exp overflow, and pads carry a -1e30 bias -> exp == 0.
                ex = work.tile([128, H, WMAX], bf16, name=f"ex{t}", tag="ex")
                nc.scalar.activation(
                    out=ex[:, :, :wt], in_=amb[:, :, :wt],
                    func=mybir.ActivationFunctionType.Exp,
                )
                den = work.tile([128, H], f32, name=f"den{t}", tag="den")
                nc.vector.tensor_reduce(
                    out=den[:], in_=ex[:, :, :wt],
                    axis=mybir.AxisListType.X, op=mybir.AluOpType.add,
                )
                rden = work.tile([128, H], f32, name=f"rden{t}", tag="rden")
                nc.vector.reciprocal(out=rden[:], in_=den[:])
                mprod = work.tile([128, H, HS, WMAX], f32, name=f"mprod{t}", tag="mprod")
                mpart = gt[:, :wt, D:].rearrange("p w (h d) -> p h d w", h=H)
                ab2 = ex[:, :, :wt].unsqueeze(2).to_broadcast([128, H, HS, wt])
                nc.vector.tensor_tensor(
                    out=mprod[:, :, :, :wt], in0=mpart, in1=ab2, op=mybir.AluOpType.mult
                )
                hm = work.tile([128, D], f32, name=f"hm{t}", tag="hm")
                nc.vector.tensor_reduce(
                    out=hm[:].rearrange("p (h d) -> p h d", h=H),
                    in_=mprod[:, :, :, :wt],
                    axis=mybir.AxisListType.X,
                    op=mybir.AluOpType.add,
                )
                hm2 = work.tile([128, D], f32, name=f"hm2{t}", tag="hm2")
                nc.vector.tensor_tensor(
                    out=hm2[:].rearrange("p (h d) -> p h d", h=H),
                    in0=hm[:].rearrange("p (h d) -> p h d", h=H),
                    in1=rden[:].unsqueeze(2).to_broadcast([128, H, HS]),
                    op=mybir.AluOpType.mult,
                )
                tp = opsum.tile([128, 128], f32, space="PSUM", name=f"tp{t}", tag="tp")
                nc.tensor.transpose(out=tp[:D, :], in_=hm2[:], identity=ident[:])
                hT = work.tile([D, 128], f32, name=f"hT{t}", tag="hT")
                nc.any.tensor_copy(out=hT[:], in_=tp[:D, :])
                o4 = opsum.tile([128, NT * D], f32, space="PSUM", name=f"o4_{t}", tag="o4")
                nc.tensor.matmul(o4[:], lhsT=hT[:], rhs=wa_s[:], start=True, stop=True)
                osel = work.tile([128, NT * D], f32, name=f"osel{t}", tag="osel")
                ohb = (
                    oneh_s[:]
                    .rearrange("p (t f) -> p t f", t=NTILES)[:, t]
                    .unsqueeze(1)
                    .to_broadcast([128, D, NT])
                )
                nc.vector.tensor_tensor(
                    out=osel[:].rearrange("p (t d) -> p d t", t=NT),
                    in0=o4[:].rearrange("p (t d) -> p d t", t=NT),
                    in1=ohb,
                    op=mybir.AluOpType.mult,
                )
                ot = work.tile([128, D], f32, name=f"ot{t}", tag="ot")
                nc.vector.tensor_reduce(
                    out=ot[:],
                    in_=osel[:].rearrange("p (t d) -> p d t", t=NT),
                    axis=mybir.AxisListType.X,
                    op=mybir.AluOpType.add,
                )
                nc.sync.dma_start(out=outp[t * 128 : (t + 1) * 128, :], in_=ot[:])

            # emission order = scheduler priority: build chunk 0's table first,
            # then interleave later chunks' table slabs with phase 3 of the
            # already-built chunks so DMA/PE/DVE overlap across phases.
            import os

            mode = os.environ.get("GNN_EMIT", "chunk")
            if mode == "chunk":
                for em in node_chunk_emitters(0):
                    em()
                late_consts()
                q_tiles(chunk_tiles[0])
                for h in range(1, NCH):
                    for em in node_chunk_emitters(h):
                        em()
                    q_tiles(chunk_tiles[h])
                    for t in chunk_tiles[h - 1]:
                        p3_tile(t)
                for t in chunk_tiles[NCH - 1]:
                    p3_tile(t)
            else:  # interleave
                for em in node_chunk_emitters(0):
                    em()
                late_consts()
                q_tiles(chunk_tiles[0])
                for h in range(1, NCH):
                    q_tiles(chunk_tiles[h])
                    slabs = node_chunk_emitters(h)
                    tiles = chunk_tiles[h - 1]
                    ns, ntl = len(slabs), len(tiles)
                    si = ti = 0
                    while si < ns or ti < ntl:
                        take = (si + 1) * ntl <= (ti + 1) * ns
                        if si < ns and (take or ti >= ntl):
                            slabs[si]()
                            si += 1
                        else:
                            p3_tile(tiles[ti])
                            ti += 1
                for t in chunk_tiles[NCH - 1]:
                    p3_tile(t)

    nc.compile()
    return nc


def kernel(x, ntype, etype, src, dst, Wk, Wq, Wv, Wa, rel_att, rel_msg, rel_pri):
    from concourse import bass_utils

    cores, consts = _host_prep(x, ntype, etype, src, dst)
    wkm, wq4, wa_all = _fold_weights(Wk, Wq, Wv, Wa, rel_att, rel_msg, rel_pri)

    struct_sig = (
        tuple(consts["W"].tolist()),
        consts["NCH"],
        tuple(consts["CHRs"]),
        tuple(consts["R"].ravel().tolist()),
    )
    if "prog" not in _cache or _cache["prog"][0] != struct_sig:
        _cache["prog"] = (struct_sig, _build_program(consts))
    nc = _cache["prog"][1]

    in_maps = [
        dict(
            xpT=d["xpT"], wkm=wkm, x4T=d["x4T"], wq4=wq4, wa=wa_all,
            oneh=d["oneh"], kmidx=d["kmidx"], abias=d["abias"],
        )
        for d in cores
    ]
    res = bass_utils.run_bass_kernel_spmd(nc, in_maps, core_ids=list(range(C)))
    global LAST_RESULT
    LAST_RESULT = res

    out = np.zeros((N, D), dtype=np.float32)
    own = consts["own_nodes"]
    for c in range(C):
        oc = res.results[c]["outp"]
        m = own[c] >= 0
        out[own[c][m]] = oc[m]
    out[consts["deg"] == 0] = 0.0
    return out

